# revision 1
# baseline (speedup 1.0000x reference)
"""Trainium2 Bass kernel for nn_DeformRouting (deformable routing conv).

Strategy (8 cores, data-parallel over N x H-halves):
  core c handles image n = c//2, row-half = c%2 (14 rows x 28 cols = 392 pixels).

Per-core device pipeline (points-on-partitions layout, 4 chunks of 98 pts):
  1. offset conv: 4 PE matmuls  out[pt,18] = x_chunk.T @ w_off.T
  2. coordinate math on [128, 36] tiles (DVE/ACT elementwise):
     grid coords -> floor, fractional weights, validity, clamped indices.
  3. bilinear gather: 2 indirect DMAs from a host-built 29x28 row-PAIR table
     (each gathered 512B row = [x[y0c], x[y0c+1]] stacked over 64 channels).
  4. combine: samp = w00*A0 + w10*A1 + w01*B0 + w11*B1  (DVE, free-dim
     broadcast weights);  q = samp * x  (the grouped weight-gen fold).
  5. PE transpose of the stacked [q; samp] tensor to (u,c)-on-partitions.
  6. 9 accumulating PE matmuls: out[o, pt] += Wstack_kk.T @ rhs_kk
     where Wstack_kk = [w_wgt_kk ; b_wgt_kk]  (the per-pixel matvec,
     algebraically refactored: out = sum_ck W2*x*samp + B2*samp).
"""

import numpy as np

import concourse.bass as bass
import concourse.tile as tile
from concourse import bacc, mybir
from concourse.bass import IndirectOffsetOnAxis
from concourse.bass_utils import run_bass_kernel_spmd
from concourse.masks import make_identity

# problem constants (hardcoded per contract)
N, CIN, COUT, H, W, K = 4, 64, 64, 28, 28, 3
K2 = K * K  # 9
NCORES = 8
HHALF = H // 2          # 14 rows per core
NPT = HHALF * W         # 392 points per core
PCH = 98                # points per partition-chunk
NCH = 4                 # chunks (4*98 = 392)
TBL_ROWS = (H + 1) * W  # 812 pair-table rows
SC = (W - 1) / 2.0      # 13.5

F32 = mybir.dt.float32
I32 = mybir.dt.int32

_CACHE = {}


def _alu(name):
    return getattr(mybir.AluOpType, name)


def _build_program():
    """Build + compile the (SPMD-identical) Bass program once."""
    nc = bacc.Bacc("TRN2", target_bir_lowering=False, debug=False,
                   num_devices=NCORES)

    # DRAM I/O (per-core shapes)
    xpair = nc.dram_tensor("xpair", [TBL_ROWS, 2 * CIN], F32, kind="ExternalInput")
    xcpad = nc.dram_tensor("xcpad", [128, NPT], F32, kind="ExternalInput")
    wofft = nc.dram_tensor("wofft", [128, 2 * K2], F32, kind="ExternalInput")
    basex = nc.dram_tensor("basex", [128, NCH * K2], F32, kind="ExternalInput")
    basey = nc.dram_tensor("basey", [128, NCH * K2], F32, kind="ExternalInput")
    wwb = nc.dram_tensor("wwb", [128, 10 * COUT], F32, kind="ExternalInput")
    mg = nc.dram_tensor("mg", [128, 8 * 128], F32, kind="ExternalInput")
    out_d = nc.dram_tensor("out", [COUT, NPT], F32, kind="ExternalOutput")

    mult, add, sub = _alu("mult"), _alu("add"), _alu("subtract")
    is_gt, is_eq = _alu("is_gt"), _alu("is_equal")
    amin, amax = _alu("min"), _alu("max")

    with tile.TileContext(nc) as tc:
        with (
            tc.tile_pool(name="const", bufs=1) as cpool,
            tc.tile_pool(name="work", bufs=1) as wpool,
            tc.tile_pool(name="psoff", bufs=1, space="PSUM") as opool,
            tc.tile_pool(name="psum", bufs=2, space="PSUM") as ppool,
            tc.tile_pool(name="pso", bufs=1, space="PSUM") as popool,
        ):
            # ---- load constants/inputs ----
            xc_sb = cpool.tile([128, NPT], F32)
            nc.sync.dma_start(xc_sb[:], xcpad.ap())
            wofft_sb = cpool.tile([128, 2 * K2], F32)
            nc.sync.dma_start(wofft_sb[:], wofft.ap())
            basex_sb = cpool.tile([128, NCH, K2], F32)
            nc.sync.dma_start(basex_sb[:], basex.ap().rearrange(
                "p (a b) -> p a b", a=NCH))
            basey_sb = cpool.tile([128, NCH, K2], F32)
            nc.sync.dma_start(basey_sb[:], basey.ap().rearrange(
                "p (a b) -> p a b", a=NCH))
            wwb_sb = cpool.tile([128, 10, COUT], F32)
            nc.sync.dma_start(wwb_sb[:], wwb.ap().rearrange(
                "p (a b) -> p a b", a=10))
            ident = cpool.tile([128, 128], F32)
            make_identity(nc, ident[:])
            mg_sb = cpool.tile([128, 8, 128], F32)
            nc.sync.dma_start(mg_sb[:], mg.ap().rearrange(
                "p (a b) -> p a b", a=8))

            # ---- 1. offset conv: psum[pt(98), ch, 18] ----
            ps_off = opool.tile([128, NCH, 2 * K2], F32)
            for ch in range(NCH):
                nc.tensor.matmul(
                    out=ps_off[:PCH, ch, :],
                    lhsT=xc_sb[:, ch * PCH:(ch + 1) * PCH],
                    rhs=wofft_sb[:],
                    start=True, stop=True,
                )
            offx = wpool.tile([128, NCH, K2], F32)
            offy = wpool.tile([128, NCH, K2], F32)
            nc.any.memset(offx[:], 0.0)
            nc.any.memset(offy[:], 0.0)
            for ch in range(NCH):
                nc.any.tensor_copy(offx[:PCH, ch, :], ps_off[:PCH, ch, 0:18:2])
                nc.any.tensor_copy(offy[:PCH, ch, :], ps_off[:PCH, ch, 1:18:2])

            # ---- 2. coordinate math on [128, 36] ----
            shp = [128, NCH, K2]
            _cnt = [0]

            def t(name=None):
                _cnt[0] += 1
                return wpool.tile(shp, F32, name=f"ct{_cnt[0]}")

            def floor_of(i_coord):
                _cnt[0] += 1
                ti = wpool.tile(shp, I32, name=f"ct{_cnt[0]}")
                nc.any.tensor_copy(ti[:], i_coord[:])     # f32 -> i32 cast
                tf = t()
                nc.any.tensor_copy(tf[:], ti[:])          # i32 -> f32 cast
                g = t()
                nc.vector.tensor_tensor(g[:], tf[:], i_coord[:], is_gt)
                f0 = t()
                nc.vector.tensor_tensor(f0[:], tf[:], g[:], sub)
                return f0

            def axis_frac(off_t, base_t):
                # i = off*13.5 + base ; returns (i, floor(i))
                i_c = t()
                nc.vector.scalar_tensor_tensor(i_c[:], off_t[:], SC, base_t[:],
                                               mult, add)
                return i_c, floor_of(i_c)

            ix, x0f = axis_frac(offx, basex_sb)
            iy, y0f = axis_frac(offy, basey_sb)

            def frac_weights(i_c, f0):
                w1 = t()
                nc.vector.tensor_tensor(w1[:], i_c[:], f0[:], sub)
                w0 = t()
                nc.vector.tensor_scalar(w0[:], w1[:], -1.0, 1.0, mult, add)
                return w0, w1

            wx0, wx1 = frac_weights(ix, x0f)
            wy0, wy1 = frac_weights(iy, y0f)

            def clip_valid(f0):
                # returns (clip(f0), valid(f0), clip(f0+1), valid(f0+1))
                c0 = t()
                nc.vector.tensor_scalar(c0[:], f0[:], 27.0, 0.0, amin, amax)
                v0 = t()
                nc.vector.tensor_tensor(v0[:], c0[:], f0[:], is_eq)
                f1 = t()
                nc.vector.tensor_scalar_add(f1[:], f0[:], 1.0)
                c1 = t()
                nc.vector.tensor_scalar(c1[:], f1[:], 27.0, 0.0, amin, amax)
                v1 = t()
                nc.vector.tensor_tensor(v1[:], c1[:], f1[:], is_eq)
                return c0, v0, c1, v1

            x0c, vx0, x1c, vx1 = clip_valid(x0f)
            _, vy0, _, vy1 = clip_valid(y0f)

            # y pair-table row: s = clip(y0f, -1, 27) + 1 ; yb = s*28
            y0cp = t()
            nc.vector.tensor_scalar(y0cp[:], y0f[:], 27.0, -1.0, amin, amax)
            yb = t()
            nc.vector.tensor_scalar(yb[:], y0cp[:], float(W), float(W), mult, add)

            def vmul(a, b):
                o = t()
                nc.vector.tensor_tensor(o[:], a[:], b[:], mult)
                return o

            wx0v, wx1v = vmul(wx0, vx0), vmul(wx1, vx1)
            wy0v, wy1v = vmul(wy0, vy0), vmul(wy1, vy1)
            w00, w01 = vmul(wy0v, wx0v), vmul(wy0v, wx1v)
            w10, w11 = vmul(wy1v, wx0v), vmul(wy1v, wx1v)

            idxa_f = t()
            nc.vector.tensor_tensor(idxa_f[:], yb[:], x0c[:], add)
            idxb_f = t()
            nc.vector.tensor_tensor(idxb_f[:], yb[:], x1c[:], add)
            # ---- 3. wrap idx into dma_gather's 16-partition layout via
            # 8 permutation matmuls: wrap[q, m*8+g] = idx_f[g*16 + q%16, m]
            NI = 128 * NCH * K2  # 4608 gathered rows per tensor

            def wrap_idx(idx_f, tag):
                psw = opool.tile([128, 8, NCH * K2], F32, tag=f"psw{tag}",
                                 name=f"psw{tag}")
                for gsel in range(8):
                    nc.tensor.matmul(
                        out=psw[:, gsel, :], lhsT=mg_sb[:, gsel, :],
                        rhs=idx_f[:].rearrange("p a b -> p (a b)"),
                        start=True, stop=True)
                wrap = wpool.tile([128, NCH * K2, 8], mybir.dt.int16,
                                  name=f"wrap{tag}")
                nc.any.tensor_copy(wrap[:].rearrange("q m g -> q g m"), psw[:])
                return wrap

            wrapa = wrap_idx(idxa_f, "a")
            wrapb = wrap_idx(idxb_f, "b")

            # ---- gathers: row i = m*128 + pt -> ga[pt, m, :] ----
            ga = wpool.tile([128, NCH, K2, 2 * CIN], F32)
            nc.gpsimd.dma_gather(
                out_ap=ga[:].rearrange("p a k c -> p (a k) c"),
                in_ap=xpair.ap(),
                idxs_ap=wrapa[:].rearrange("q m g -> q (m g)"),
                num_idxs=NI, num_idxs_reg=NI, elem_size=2 * CIN,
                single_packet=False)
            gb = wpool.tile([128, NCH, K2, 2 * CIN], F32)
            nc.gpsimd.dma_gather(
                out_ap=gb[:].rearrange("p a k c -> p (a k) c"),
                in_ap=xpair.ap(),
                idxs_ap=wrapb[:].rearrange("q m g -> q (m g)"),
                num_idxs=NI, num_idxs_reg=NI, elem_size=2 * CIN,
                single_packet=False)

            # ---- 4. combine ----
            def bc(wt):
                return wt[:, :, :, None].to_broadcast([128, NCH, K2, CIN])

            samp_t = wpool.tile([128, NCH, CIN, K2], F32)  # m=(c,kk) inner
            samp = samp_t[:].rearrange("p a c k -> p a k c")
            tmp_t = wpool.tile([128, NCH, CIN, K2], F32)
            tmp = tmp_t[:].rearrange("p a c k -> p a k c")
            nc.vector.tensor_tensor(samp, ga[:, :, :, 0:CIN], bc(w00), mult)
            nc.vector.tensor_tensor(tmp, ga[:, :, :, CIN:], bc(w10), mult)
            nc.vector.tensor_tensor(samp, samp, tmp, add)
            nc.vector.tensor_tensor(tmp, gb[:, :, :, 0:CIN], bc(w01), mult)
            nc.vector.tensor_tensor(samp, samp, tmp, add)
            nc.vector.tensor_tensor(tmp, gb[:, :, :, CIN:], bc(w11), mult)
            nc.vector.tensor_tensor(samp, samp, tmp, add)

            # ---- 5. transpose to s-chunks [m(128), pt] ----
            NB = 5  # ceil(576/128)
            rhs = wpool.tile([128, NB, NPT], F32)
            # rows 64:128 of the last m-chunk are padding (576 -> 640): the
            # K=128 matmul reads them, so they must be zeroed (their weights
            # are zero, but NaN garbage would still poison the product).
            nc.any.memset(rhs[64:, NB - 1, :], 0.0)
            sv = samp_t[:].rearrange("p a c k -> p a (c k)")
            for ch in range(NCH):
                for b in range(NB):
                    mlo, mhi = 128 * b, min(128 * (b + 1), CIN * K2)
                    pst = ppool.tile([128, 128], F32, tag="tps")
                    nc.tensor.transpose(
                        pst[:mhi - mlo, :], sv[:, ch, mlo:mhi], ident[:])
                    nc.any.tensor_copy(
                        rhs[:mhi - mlo, b, ch * PCH:(ch + 1) * PCH],
                        pst[:mhi - mlo, :PCH])

            # ---- 6. final matmuls: ps1 = W~ @ s, ps2 = B~ @ s ----
            ps1 = popool.tile([COUT, NPT], F32, name="ps1")
            ps2 = popool.tile([COUT, NPT], F32, name="ps2")
            for b in range(NB):
                nc.tensor.matmul(
                    out=ps1[:], lhsT=wwb_sb[:, b, :], rhs=rhs[:, b, :],
                    start=(b == 0), stop=(b == NB - 1))
            for b in range(NB):
                nc.tensor.matmul(
                    out=ps2[:], lhsT=wwb_sb[:, NB + b, :], rhs=rhs[:, b, :],
                    start=(b == 0), stop=(b == NB - 1))
            out_sb = wpool.tile([COUT, NPT], F32)
            nc.vector.tensor_tensor(out_sb[:], ps1[:], xc_sb[:COUT, :], mult)
            nc.vector.tensor_tensor(out_sb[:], out_sb[:], ps2[:], add)
            nc.sync.dma_start(out_d.ap(), out_sb[:])

    nc.compile()
    return nc


def _host_inputs(x, w_off, b_off, w_wgt, b_wgt):
    """Build the 8 per-core input dicts (layout/shard prep only)."""
    x = np.asarray(x, dtype=np.float32)
    w_off = np.asarray(w_off, dtype=np.float32)
    b_off = np.asarray(b_off, dtype=np.float32)
    w_wgt = np.asarray(w_wgt, dtype=np.float32)
    b_wgt = np.asarray(b_wgt, dtype=np.float32)

    xs = np.linspace(-1.0, 1.0, W).astype(np.float32)
    ys = np.linspace(-1.0, 1.0, H).astype(np.float32)
    kx = np.linspace(-(K - 1) / (W - 1), (K - 1) / (W - 1), K).astype(np.float32)
    ky = np.linspace(-(K - 1) / (H - 1), (K - 1) / (H - 1), K).astype(np.float32)

    # wwb [128, 10, 64]: chunks 0..4 = W~.T (640x64, zero-padded from 576),
    # chunks 5..9 = B~.T, where W~ = w_wgt [64, 576], B~ = b_wgt.reshape(64, 576)
    wtp = np.zeros((640, COUT), dtype=np.float32)
    wtp[:576] = w_wgt.T
    btp = np.zeros((640, COUT), dtype=np.float32)
    btp[:576] = b_wgt.reshape(CIN, K2 * COUT).T
    wwb = np.concatenate([wtp.reshape(5, 128, COUT),
                          btp.reshape(5, 128, COUT)], axis=0)  # [10,128,64]
    wwb = wwb.transpose(1, 0, 2).reshape(128, 10 * COUT).copy()

    # idx-wrap permutation selectors: mg[pt, g*128+q] = (pt == g*16 + q%16)
    mg = np.zeros((128, 8, 128), dtype=np.float32)
    q = np.arange(128)
    for gsel in range(8):
        mg[gsel * 16 + (q % 16), gsel, q] = 1.0
    mg = mg.reshape(128, 8 * 128)

    wofft = np.zeros((128, 2 * K2), dtype=np.float32)
    wofft[:CIN] = w_off.T

    in_maps = []
    for c in range(NCORES):
        n, half = divmod(c, 2)
        r0 = HHALF * half
        xn = x[n]                             # [64, 28, 28]
        x_hwc = xn.transpose(1, 2, 0)         # [28, 28, 64]

        tbl = np.zeros((H + 1, W, 2 * CIN), dtype=np.float32)
        rt = np.clip(np.arange(H + 1) - 1, 0, H - 1)
        rb = np.clip(np.arange(H + 1), 0, H - 1)
        tbl[:, :, :CIN] = x_hwc[rt]
        tbl[:, :, CIN:] = x_hwc[rb]

        xcpad = np.zeros((128, NPT), dtype=np.float32)
        xcpad[:CIN] = xn.reshape(CIN, H * W)[:, r0 * W:r0 * W + NPT]

        # base grids [128, NCH, K2]
        bx = np.zeros((128, NCH, K2), dtype=np.float32)
        by = np.zeros((128, NCH, K2), dtype=np.float32)
        p_idx = np.arange(PCH)
        for ch in range(NCH):
            g = r0 * W + ch * PCH + p_idx          # global pixel
            row, col = g // W, g % W
            for kk in range(K2):
                kyi, kxi = divmod(kk, K)
                bx[:PCH, ch, kk] = (xs[col] + kx[kxi] + b_off[2 * kk] + 1.0) * SC
                by[:PCH, ch, kk] = (ys[row] + ky[kyi] + b_off[2 * kk + 1] + 1.0) * SC
        # pad rows: safe in-range coords (center pixel, zero offset)
        bx[PCH:] = SC
        by[PCH:] = SC

        in_maps.append({
            "xpair": tbl.reshape(TBL_ROWS, 2 * CIN),
            "xcpad": xcpad,
            "wofft": wofft,
            "basex": bx.reshape(128, NCH * K2),
            "basey": by.reshape(128, NCH * K2),
            "wwb": wwb,
            "mg": mg,
        })
    return in_maps


def get_program():
    if "nc" not in _CACHE:
        _CACHE["nc"] = _build_program()
    return _CACHE["nc"]


def run_cores(in_maps, **kw):
    nc = get_program()
    return run_bass_kernel_spmd(nc, in_maps, core_ids=list(range(NCORES)), **kw)


def assemble(results):
    out = np.zeros((N, COUT, H, W), dtype=np.float32)
    for c in range(NCORES):
        n, half = divmod(c, 2)
        out[n, :, HHALF * half:HHALF * (half + 1), :] = \
            results[c]["out"].reshape(COUT, HHALF, W)
    return out


def kernel(x, w_off, b_off, w_wgt, b_wgt):
    in_maps = _host_inputs(x, w_off, b_off, w_wgt, b_wgt)
    res = run_cores(in_maps)
    return assemble(res.results)



# revision 6
# speedup vs baseline: 2.0609x; 2.0609x over previous
"""Trainium2 Bass kernel for nn_DeformRouting (deformable routing conv), v2.

Strategy (8 cores, data-parallel over N x H-halves):
  core c handles image n = c//2, row-half = c%2 (14 rows x 28 cols = 392 pixels).

Key design points vs v1 (181us -> target ~55us):
  - ONE dma_gather instead of two: a 4-tap table row (y0,x0) holds all four
    bilinear neighbors [v00|v01|v10|v11] x 64ch in bf16 (512B rows).  The
    table is zero-padded over a 31x31 clipped grid, so out-of-bounds taps
    read zeros and NO validity masking is needed on-device.
  - Gather split in two (18+18 m-slots) so chunk-A combine/transpose work
    hides under chunk-B's descriptor generation (the dominant serial cost,
    ~9.5ns/idx of gpsimd ucode).
  - Coordinate math runs on fused x|y tiles ([128, ch, 18]) in fp32:
    i' = conv*13.5 + base ; floor ; frac ; clip(0,30); row = 31*cy+cx.
  - Contraction order is k-major (m = k*64 + c, host-permuted weights), so
    the tap-combine writes samp[q, ch, k, c] fully contiguously in bf16.
  - W~ and B~ are stacked in one [128,128] lhsT per m-chunk: 5 accumulating
    bf16 matmuls produce [ps1; ps2] = [W~@s ; B~@s] in one PSUM tile.
  - out = ps1 * x + ps2 in fp32 (the per-pixel grouped weight-gen algebra:
    weight[o,c',k] = x[o]*w_wgt[o,c'k] + b_wgt[o,c'k]).
"""

import numpy as np

import concourse.bass as bass
import concourse.tile as tile
from concourse import bacc, mybir
from concourse.bass_utils import run_bass_kernel_spmd
from concourse.masks import make_identity

# problem constants (hardcoded per contract)
N, CIN, COUT, H, W, K = 4, 64, 64, 28, 28, 3
K2 = K * K  # 9
NCORES = 8
HHALF = H // 2          # 14 rows per core
NPT = HHALF * W         # 392 points per core
PCH = 98                # points per partition-chunk
NCH = 4                 # chunks (4*98 = 392)
TBL_ROWS = 31 * 31      # 961 4-tap table rows (clipped 31x31 grid)
SC = (W - 1) / 2.0      # 13.5
NSPLIT = 2              # gather split (ch-pairs)
MSLOT = NCH * K2        # 36 m-slots (ch, k)
NI_HALF = 128 * (MSLOT // NSPLIT)  # 2304 gathered rows per half

F32 = mybir.dt.float32
I32 = mybir.dt.int32
BF16 = mybir.dt.bfloat16

_CACHE = {}


def _alu(name):
    return getattr(mybir.AluOpType, name)


def _build_program():
    nc = bacc.Bacc("TRN2", target_bir_lowering=False, debug=False,
                   num_devices=NCORES)

    # DRAM I/O (per-core shapes)
    tbl4 = nc.dram_tensor("tbl4", [TBL_ROWS, 4 * CIN], BF16, kind="ExternalInput")
    xcpad = nc.dram_tensor("xcpad", [128, NPT], F32, kind="ExternalInput")
    wofft = nc.dram_tensor("wofft", [128, 2 * K2], F32, kind="ExternalInput")
    base2 = nc.dram_tensor("base2", [128, NCH * 2 * K2], F32, kind="ExternalInput")
    wwb = nc.dram_tensor("wwb", [128, 5 * 128], BF16, kind="ExternalInput")
    mg = nc.dram_tensor("mg", [128, 8 * 128], F32, kind="ExternalInput")
    out_d = nc.dram_tensor("out", [COUT, NPT], F32, kind="ExternalOutput")

    mult, add, sub = _alu("mult"), _alu("add"), _alu("subtract")
    is_gt = _alu("is_gt")
    amin, amax = _alu("min"), _alu("max")

    with tile.TileContext(nc) as tc:
        with (
            tc.tile_pool(name="const", bufs=1) as cpool,
            tc.tile_pool(name="work", bufs=1) as wpool,
            tc.tile_pool(name="psoff", bufs=1, space="PSUM") as opool,
            tc.tile_pool(name="pst", bufs=2, space="PSUM") as tpool,
            tc.tile_pool(name="pso", bufs=1, space="PSUM") as popool,
        ):
            # ---- load constants/inputs ----
            xc_sb = cpool.tile([128, NPT], F32)
            nc.sync.dma_start(xc_sb[:], xcpad.ap())
            wofft_sb = cpool.tile([128, 2 * K2], F32)
            nc.sync.dma_start(wofft_sb[:], wofft.ap())
            base2_sb = cpool.tile([128, NCH, 2 * K2], F32)
            nc.sync.dma_start(base2_sb[:], base2.ap().rearrange(
                "p (a b) -> p a b", a=NCH))
            wwb_sb = cpool.tile([128, 5, 128], BF16)
            nc.sync.dma_start(wwb_sb[:], wwb.ap().rearrange(
                "p (a b) -> p a b", a=5))
            mg_sb = cpool.tile([128, 8, 128], F32)
            nc.sync.dma_start(mg_sb[:], mg.ap().rearrange(
                "p (a b) -> p a b", a=8))
            identb = cpool.tile([128, 128], BF16)
            make_identity(nc, identb[:])

            # ---- 1. offset conv: ps_off[pt(98), ch, 18] ----
            # wofft columns are host-permuted to [9 x-offsets | 9 y-offsets].
            # Pad partitions (98:128) must be zeroed: garbage would flow into
            # the gather indices (clip would not catch NaN).
            ps_off = opool.tile([128, NCH, 2 * K2], F32)
            nc.any.memset(ps_off[:], 0.0)
            for ch in range(NCH):
                nc.tensor.matmul(
                    out=ps_off[:PCH, ch, :],
                    lhsT=xc_sb[:, ch * PCH:(ch + 1) * PCH],
                    rhs=wofft_sb[:],
                    start=True, stop=True,
                )

            # ---- 2. coordinate math on fused x|y tiles [128, NCH, 18] ----
            shp2 = [128, NCH, 2 * K2]
            shp1 = [128, NCH, K2]
            _cnt = [0]

            def t(shape=shp2, dt=F32):
                _cnt[0] += 1
                return wpool.tile(shape, dt, name=f"ct{_cnt[0]}")

            # i' = conv*13.5 + base  (base includes +2 pad, +b_off*13.5, and
            # pad-partition safe coords)
            ic = t()
            nc.vector.scalar_tensor_tensor(ic[:], ps_off[:], SC, base2_sb[:],
                                           mult, add)
            # f0 = floor(i')
            ti = t(dt=I32)
            nc.any.tensor_copy(ti[:], ic[:])
            tf = t()
            nc.any.tensor_copy(tf[:], ti[:])
            g = t()
            nc.vector.tensor_tensor(g[:], tf[:], ic[:], is_gt)
            f0 = t()
            nc.vector.tensor_tensor(f0[:], tf[:], g[:], sub)
            # fractional weights
            w1 = t()
            nc.vector.tensor_tensor(w1[:], ic[:], f0[:], sub)
            w0 = t()
            nc.vector.tensor_scalar(w0[:], w1[:], -1.0, 1.0, mult, add)
            # clipped table coords
            cc = t()
            nc.vector.tensor_scalar(cc[:], f0[:], 30.0, 0.0, amin, amax)
            # row = 31*cy + cx
            idxf = t(shp1)
            nc.vector.scalar_tensor_tensor(idxf[:], cc[:, :, K2:], 31.0,
                                           cc[:, :, :K2], mult, add)
            # tap weight products (bf16 for the combine)
            wx0, wx1 = w0[:, :, :K2], w1[:, :, :K2]
            wy0, wy1 = w0[:, :, K2:], w1[:, :, K2:]
            w4 = t([128, 4, NCH, K2], BF16)
            nc.vector.tensor_tensor(w4[:, 0], wy0, wx0, mult)
            nc.vector.tensor_tensor(w4[:, 1], wy0, wx1, mult)
            nc.vector.tensor_tensor(w4[:, 2], wy1, wx0, mult)
            nc.vector.tensor_tensor(w4[:, 3], wy1, wx1, mult)

            # ---- 3. wrap idx into dma_gather's 16-partition layout via
            # 8 permutation matmuls: wrap[q, m, g] = idxf[g*16 + q%16, m]
            psw = opool.tile([128, 8, MSLOT], F32, name="psw")
            for gsel in range(8):
                nc.tensor.matmul(
                    out=psw[:, gsel, :], lhsT=mg_sb[:, gsel, :],
                    rhs=idxf[:].rearrange("p a b -> p (a b)"),
                    start=True, stop=True)
            wrap = wpool.tile([128, MSLOT, 8], mybir.dt.int16, name="wrap")
            nc.any.tensor_copy(wrap[:].rearrange("q m g -> q g m"), psw[:])

            # ---- 4+5+6. per-half: gather -> combine -> transpose ----
            MH = MSLOT // NSPLIT  # 18 m-slots per half
            NB = 5  # m-chunks of 128 (576 -> 640)
            rhs = wpool.tile([128, NB, NPT], BF16)
            samp = wpool.tile([128, NCH, K2, CIN], BF16)

            def bc(wt):
                return wt[:, :, :, None].to_broadcast([128, NCH // NSPLIT,
                                                       K2, CIN])

            for hf in range(NSPLIT):
                ch0 = hf * (NCH // NSPLIT)
                ga = wpool.tile([128, MH, 4, CIN], BF16, name=f"ga{hf}",
                                tag=f"ga{hf}")
                nc.gpsimd.dma_gather(
                    out_ap=ga[:].rearrange("p m t c -> p m (t c)"),
                    in_ap=tbl4.ap(),
                    idxs_ap=wrap[:, hf * MH:(hf + 1) * MH, :].rearrange(
                        "q m g -> q (m g)"),
                    num_idxs=NI_HALF, num_idxs_reg=NI_HALF,
                    elem_size=4 * CIN, single_packet=False)
                gav = ga[:].rearrange("p (a k) t c -> p a k t c",
                                      a=NCH // NSPLIT)
                sv = samp[:, ch0:ch0 + NCH // NSPLIT]
                wv = w4[:, :, ch0:ch0 + NCH // NSPLIT]
                tmp = wpool.tile([128, NCH // NSPLIT, K2, CIN], BF16,
                                 name=f"tmp{hf}", tag=f"tmp{hf}")
                # samp = sum_t w_t * ga_t  (DVE + scalar engine split)
                nc.vector.tensor_tensor(sv, gav[:, :, :, 0], bc(wv[:, 0]), mult)
                nc.vector.tensor_tensor(tmp[:], gav[:, :, :, 1], bc(wv[:, 1]),
                                        mult)
                nc.vector.tensor_tensor(sv, sv, tmp[:], add)
                nc.vector.tensor_tensor(tmp[:], gav[:, :, :, 2], bc(wv[:, 2]),
                                        mult)
                nc.vector.tensor_tensor(sv, sv, tmp[:], add)
                nc.vector.tensor_tensor(tmp[:], gav[:, :, :, 3], bc(wv[:, 3]),
                                        mult)
                nc.vector.tensor_tensor(sv, sv, tmp[:], add)

                # transpose samp[q, ch, (k c)] -> rhs[(k c), b, pt]
                sf = samp[:].rearrange("p a k c -> p a (k c)")
                for ch in range(ch0, ch0 + NCH // NSPLIT):
                    for b in range(NB):
                        mlo, mhi = 128 * b, min(128 * (b + 1), CIN * K2)
                        pst = tpool.tile([128, 128], BF16, tag="tps")
                        nc.tensor.transpose(
                            pst[:mhi - mlo, :], sf[:, ch, mlo:mhi], identb[:])
                        nc.any.tensor_copy(
                            rhs[:mhi - mlo, b, ch * PCH:(ch + 1) * PCH],
                            pst[:mhi - mlo, :PCH])

            # zero pad rows 64:128 of the last m-chunk (weights there are 0,
            # but NaN garbage would still poison the product)
            nc.any.memset(rhs[64:, NB - 1, :], 0.0)

            # ---- 7. stacked matmuls: psWB = [W~ ; B~] @ s ----
            psWB = popool.tile([128, NPT], F32, name="psWB")
            for b in range(NB):
                nc.tensor.matmul(
                    out=psWB[:], lhsT=wwb_sb[:, b, :], rhs=rhs[:, b, :],
                    start=(b == 0), stop=(b == NB - 1))
            out_sb = wpool.tile([COUT, NPT], F32)
            nc.vector.tensor_tensor(out_sb[:], psWB[:COUT, :],
                                    xc_sb[:COUT, :], mult)
            nc.vector.tensor_tensor(out_sb[:], out_sb[:], psWB[COUT:, :], add)
            nc.sync.dma_start(out_d.ap(), out_sb[:])

    nc.compile()
    return nc


def _host_inputs(x, w_off, b_off, w_wgt, b_wgt):
    """Build the 8 per-core input dicts (layout/shard prep only)."""
    x = np.asarray(x, dtype=np.float32)
    w_off = np.asarray(w_off, dtype=np.float32)
    b_off = np.asarray(b_off, dtype=np.float32)
    w_wgt = np.asarray(w_wgt, dtype=np.float32)
    b_wgt = np.asarray(b_wgt, dtype=np.float32)

    # wwb [128, 5, 128]: lhsT chunk b = [W~.T rows | B~.T rows] stacked on
    # the output axis, with k-major contraction order m = k*64 + c.
    perm = np.arange(CIN * K2).reshape(CIN, K2).T.reshape(-1)  # m -> c*9+k
    wtp = np.zeros((640, COUT), dtype=np.float32)
    wtp[:576] = w_wgt.T[perm]
    btp = np.zeros((640, COUT), dtype=np.float32)
    btp[:576] = b_wgt.reshape(CIN, K2 * COUT).T[perm]
    wwb = np.concatenate([wtp.reshape(5, 128, COUT),
                          btp.reshape(5, 128, COUT)], axis=2)  # [5,128,128]
    wwb = wwb.transpose(1, 0, 2).reshape(128, 5 * 128)
    wwb_b = _to_bf16(np.ascontiguousarray(wwb))

    # idx-wrap permutation selectors: mg[pt, g*128+q] = (pt == g*16 + q%16)
    mg = np.zeros((128, 8, 128), dtype=np.float32)
    q = np.arange(128)
    for gsel in range(8):
        mg[gsel * 16 + (q % 16), gsel, q] = 1.0
    mg = mg.reshape(128, 8 * 128)

    # offset conv weights, columns permuted to [x-offsets(9) | y-offsets(9)]
    wofft = np.zeros((128, 2 * K2), dtype=np.float32)
    wofft[:CIN, :K2] = w_off[0::2].T     # x offsets of taps 0..8
    wofft[:CIN, K2:] = w_off[1::2].T     # y offsets

    # base grids ([x|y] fused), in padded table coords (+2), b_off folded in
    xs = np.linspace(-1.0, 1.0, W).astype(np.float32)
    ys = np.linspace(-1.0, 1.0, H).astype(np.float32)
    kx = np.linspace(-(K - 1) / (W - 1), (K - 1) / (W - 1), K).astype(np.float32)
    ky = np.linspace(-(K - 1) / (H - 1), (K - 1) / (H - 1), K).astype(np.float32)

    in_maps = []
    for c in range(NCORES):
        n, half = divmod(c, 2)
        r0 = HHALF * half
        xn = x[n]                             # [64, 28, 28]

        # 4-tap table over the clipped 31x31 grid: row (cy, cx) holds the
        # four neighbors of integer corner (y0, x0) = (cy-2, cx-2), with
        # out-of-bounds taps = 0.
        pad = np.zeros((CIN, H + 5, W + 5), dtype=np.float32)
        pad[:, 2:2 + H, 2:2 + W] = xn
        # taps at (y0, x0) -> pad coords (cy, cx), (cy, cx+1), (cy+1, cx),
        # (cy+1, cx+1)
        t00 = pad[:, 0:31, 0:31]
        t01 = pad[:, 0:31, 1:32]
        t10 = pad[:, 1:32, 0:31]
        t11 = pad[:, 1:32, 1:32]
        tbl = np.stack([t00, t01, t10, t11], axis=0)  # [4, 64, 31, 31]
        tbl = tbl.transpose(2, 3, 0, 1).reshape(TBL_ROWS, 4 * CIN)
        tbl_b = _to_bf16(np.ascontiguousarray(tbl))

        xcpad = np.zeros((128, NPT), dtype=np.float32)
        xcpad[:CIN] = xn.reshape(CIN, H * W)[:, r0 * W:r0 * W + NPT]

        # fused base grid [128, NCH, 18] = [x taps 0..8 | y taps 0..8]
        b2 = np.zeros((128, NCH, 2 * K2), dtype=np.float32)
        p_idx = np.arange(PCH)
        for ch in range(NCH):
            gpix = r0 * W + ch * PCH + p_idx
            row, col = gpix // W, gpix % W
            for kk in range(K2):
                kyi, kxi = divmod(kk, K)
                b2[:PCH, ch, kk] = ((xs[col] + kx[kxi] + b_off[2 * kk] + 1.0)
                                    * SC + 2.0)
                b2[:PCH, ch, K2 + kk] = ((ys[row] + ky[kyi] + b_off[2 * kk + 1]
                                          + 1.0) * SC + 2.0)
        # pad partitions: safe in-range coords (center-ish)
        b2[PCH:] = SC + 2.0

        in_maps.append({
            "tbl4": tbl_b,
            "xcpad": xcpad,
            "wofft": wofft,
            "base2": b2.reshape(128, NCH * 2 * K2),
            "wwb": wwb_b,
            "mg": mg,
        })
    return in_maps


def _to_bf16(a):
    """float32 -> bfloat16 (round-to-nearest-even), via ml_dtypes if present
    else uint16 view trick."""
    try:
        import ml_dtypes
        return a.astype(ml_dtypes.bfloat16)
    except ImportError:
        b = a.view(np.uint32)
        rounded = ((b + 0x7FFF + ((b >> 16) & 1)) >> 16).astype(np.uint16)
        return rounded.view(np.uint16)


def get_program():
    if "nc" not in _CACHE:
        _CACHE["nc"] = _build_program()
    return _CACHE["nc"]


def run_cores(in_maps, **kw):
    nc = get_program()
    return run_bass_kernel_spmd(nc, in_maps, core_ids=list(range(NCORES)), **kw)


def assemble(results):
    out = np.zeros((N, COUT, H, W), dtype=np.float32)
    for c in range(NCORES):
        n, half = divmod(c, 2)
        out[n, :, HHALF * half:HHALF * (half + 1), :] = \
            results[c]["out"].reshape(COUT, HHALF, W)
    return out


def kernel(x, w_off, b_off, w_wgt, b_wgt):
    in_maps = _host_inputs(x, w_off, b_off, w_wgt, b_wgt)
    res = run_cores(in_maps)
    return assemble(res.results)


# revision 9
# speedup vs baseline: 2.3288x; 1.1300x over previous
"""Trainium2 Bass kernel for nn_DeformRouting (deformable routing conv), v3.

Strategy (8 cores, data-parallel over N x H-halves):
  core c handles image n = c//2, row-half = c%2 (14 rows x 28 cols = 392 pixels).

v3 structure (v2 was 88us, baseline 181us):
  - 4-tap bf16 table rows (zero-padded 31x31 grid) -> ONE gather index per
    sample, validity masking folded into table zeros.
  - Per-chunk pipeline (4 chunks of 98 points): gather -> combine ->
    transpose -> column-slice matmuls, so only the LAST chunk's tail
    (drain+combine+transpose+matmul) is exposed behind the serial
    descriptor-generation (the dominant cost, ~8ns/idx of gpsimd ucode).
  - f32r (tf32-mode) offset-conv + index-permutation matmuls (values are
    small integers -> exact; avoids fp32's 2-instruction split).
  - Input DMAs spread across engine queues.
  - Last chunk's combine splits muls DVE/gpsimd (gpsimd is idle once the
    last gather's descriptors are generated).
  - k-major contraction (m = k*64+c, host-permuted weights): contiguous
    bf16 combine writes; W~ and B~ stacked in one [128,128] lhsT chunk so 5
    matmuls per column-slice produce [W~@s ; B~@s] together.
"""

import numpy as np

import concourse.bass as bass
import concourse.tile as tile
from concourse import bacc, mybir
from concourse.bass_utils import run_bass_kernel_spmd
from concourse.masks import make_identity

N, CIN, COUT, H, W, K = 4, 64, 64, 28, 28, 3
K2 = K * K
NCORES = 8
HHALF = H // 2          # 14 rows per core
NPT = HHALF * W         # 392 points per core
PCH = 98                # points per partition-chunk
NCH = 4                 # chunks
TBL_ROWS = 31 * 31      # 961 4-tap table rows
SC = (W - 1) / 2.0      # 13.5
NI_CH = 128 * K2        # 1152 gathered rows per chunk
NB = 5                  # m-chunks of 128 (576 -> 640)

F32 = mybir.dt.float32
F32R = mybir.dt.float32r
I32 = mybir.dt.int32
BF16 = mybir.dt.bfloat16

_CACHE = {}


def _alu(name):
    return getattr(mybir.AluOpType, name)


def _build_program():
    nc = bacc.Bacc("TRN2", target_bir_lowering=False, debug=False,
                   num_devices=NCORES)

    tbl4 = nc.dram_tensor("tbl4", [TBL_ROWS, 4 * CIN], BF16, kind="ExternalInput")
    xcpad = nc.dram_tensor("xcpad", [128, NPT], F32, kind="ExternalInput")
    wofft = nc.dram_tensor("wofft", [128, 2 * K2], F32, kind="ExternalInput")
    base2 = nc.dram_tensor("base2", [128, NCH * 2 * K2], F32, kind="ExternalInput")
    wwb = nc.dram_tensor("wwb", [128, NB * 128], BF16, kind="ExternalInput")
    mg = nc.dram_tensor("mg", [128, 8 * 128], F32, kind="ExternalInput")
    out_d = nc.dram_tensor("out", [COUT, NPT], F32, kind="ExternalOutput")

    mult, add, sub = _alu("mult"), _alu("add"), _alu("subtract")
    is_gt = _alu("is_gt")
    amin, amax = _alu("min"), _alu("max")

    with tile.TileContext(nc) as tc:
        with (
            tc.tile_pool(name="const", bufs=1) as cpool,
            tc.tile_pool(name="work", bufs=1) as wpool,
            tc.tile_pool(name="psoff", bufs=1, space="PSUM") as opool,
            tc.tile_pool(name="pst", bufs=2, space="PSUM") as tpool,
            tc.tile_pool(name="pso", bufs=1, space="PSUM") as popool,
        ):
            # ---- inputs, spread across engine DMA queues ----
            xc_sb = cpool.tile([128, NPT], F32)
            nc.sync.dma_start(xc_sb[:], xcpad.ap())
            wofft_sb = cpool.tile([128, 2 * K2], F32)
            nc.scalar.dma_start(wofft_sb[:], wofft.ap())
            base2_sb = cpool.tile([128, NCH, 2 * K2], F32)
            nc.sync.dma_start(base2_sb[:], base2.ap().rearrange(
                "p (a b) -> p a b", a=NCH))
            wwb_sb = cpool.tile([128, NB, 128], BF16)
            nc.scalar.dma_start(wwb_sb[:], wwb.ap().rearrange(
                "p (a b) -> p a b", a=NB))
            mg_sb = cpool.tile([128, 8, 128], F32)
            nc.sync.dma_start(mg_sb[:], mg.ap().rearrange(
                "p (a b) -> p a b", a=8))
            identb = cpool.tile([128, 128], BF16)
            make_identity(nc, identb[:])

            # ---- 1. offset conv (f32r; pad partitions zeroed: garbage
            # would flow into gather indices past the clip) ----
            ps_off = opool.tile([128, NCH, 2 * K2], F32)
            nc.any.memset(ps_off[:], 0.0)
            for ch in range(NCH):
                nc.tensor.matmul(
                    out=ps_off[:PCH, ch, :],
                    lhsT=xc_sb[:, ch * PCH:(ch + 1) * PCH],
                    rhs=wofft_sb[:],
                    start=True, stop=True,
                )

            # ---- 2. coordinate math on fused x|y tiles [128, NCH, 18] ----
            shp2 = [128, NCH, 2 * K2]
            _cnt = [0]

            def t(shape=shp2, dt=F32):
                _cnt[0] += 1
                return wpool.tile(shape, dt, name=f"ct{_cnt[0]}")

            ic = t()
            nc.vector.scalar_tensor_tensor(ic[:], ps_off[:], SC, base2_sb[:],
                                           mult, add)
            ti = t(dt=I32)
            nc.any.tensor_copy(ti[:], ic[:])
            tf = t()
            nc.any.tensor_copy(tf[:], ti[:])
            g = t()
            nc.vector.tensor_tensor(g[:], tf[:], ic[:], is_gt)
            f0 = t()
            nc.vector.tensor_tensor(f0[:], tf[:], g[:], sub)
            w1 = t()
            nc.vector.tensor_tensor(w1[:], ic[:], f0[:], sub)
            w0 = t()
            nc.vector.tensor_scalar(w0[:], w1[:], -1.0, 1.0, mult, add)
            cc = t()
            nc.vector.tensor_scalar(cc[:], f0[:], 30.0, 0.0, amin, amax)
            idxf = t([128, NCH, K2])
            nc.vector.scalar_tensor_tensor(idxf[:], cc[:, :, K2:], 31.0,
                                           cc[:, :, :K2], mult, add)
            w4 = t([128, 4, NCH, K2], BF16)
            nc.vector.tensor_tensor(w4[:, 0], w0[:, :, K2:], w0[:, :, :K2], mult)
            nc.vector.tensor_tensor(w4[:, 1], w0[:, :, K2:], w1[:, :, :K2], mult)
            nc.vector.tensor_tensor(w4[:, 2], w1[:, :, K2:], w0[:, :, :K2], mult)
            nc.vector.tensor_tensor(w4[:, 3], w1[:, :, K2:], w1[:, :, :K2], mult)

            # ---- 3. idx wrap: 8 f32r permutation matmuls + int16 copy ----
            psw = opool.tile([128, 8, NCH * K2], F32, name="psw")
            for gsel in range(8):
                nc.tensor.matmul(
                    out=psw[:, gsel, :], lhsT=mg_sb[:, gsel, :],
                    rhs=idxf[:].rearrange("p a b -> p (a b)"),
                    start=True, stop=True)
            wrap = wpool.tile([128, NCH, K2, 8], mybir.dt.int16, name="wrap")
            for ch in range(NCH):
                nc.vector.tensor_copy(
                    wrap[:, ch].rearrange("q m g -> q g m"),
                    psw[:, :, ch * K2:(ch + 1) * K2])

            # ---- 4..7 per-chunk pipeline ----
            psWB = popool.tile([128, NPT], F32, name="psWB")
            rhs = wpool.tile([128, NB, NPT], BF16)
            nc.any.memset(rhs[64:, NB - 1, :], 0.0)
            out_sb = wpool.tile([COUT, NPT], F32)

            for ch in range(NCH):
                cs = slice(ch * PCH, (ch + 1) * PCH)
                ga = wpool.tile([128, K2, 4, CIN], BF16, name=f"ga{ch}")
                nc.gpsimd.dma_gather(
                    out_ap=ga[:].rearrange("p k t c -> p k (t c)"),
                    in_ap=tbl4.ap(),
                    idxs_ap=wrap[:, ch].rearrange("q m g -> q (m g)"),
                    num_idxs=NI_CH, num_idxs_reg=NI_CH,
                    elem_size=4 * CIN, single_packet=False)

                def bcw(tap):
                    return w4[:, tap, ch][:, :, None].to_broadcast(
                        [128, K2, CIN])

                samp = wpool.tile([128, K2, CIN], BF16, name=f"samp{ch}")
                tmp = wpool.tile([128, K2, CIN], BF16, name=f"tmp{ch}")
                if ch < NCH - 1:
                    nc.vector.tensor_tensor(samp[:], ga[:, :, 0], bcw(0), mult)
                    nc.vector.tensor_tensor(tmp[:], ga[:, :, 1], bcw(1), mult)
                    nc.vector.tensor_tensor(samp[:], samp[:], tmp[:], add)
                    nc.vector.tensor_tensor(tmp[:], ga[:, :, 2], bcw(2), mult)
                    nc.vector.tensor_tensor(samp[:], samp[:], tmp[:], add)
                    nc.vector.tensor_tensor(tmp[:], ga[:, :, 3], bcw(3), mult)
                    nc.vector.tensor_tensor(samp[:], samp[:], tmp[:], add)
                else:
                    # last chunk: gpsimd is idle after its descriptor gen --
                    # split the muls so the exposed tail shrinks
                    tmp2 = wpool.tile([128, K2, CIN], BF16, name="tmpg")
                    nc.vector.tensor_tensor(samp[:], ga[:, :, 0], bcw(0), mult)
                    nc.vector.tensor_tensor(tmp[:], ga[:, :, 1], bcw(1), mult)
                    nc.gpsimd.tensor_tensor(tmp2[:], ga[:, :, 2], bcw(2), mult)
                    nc.vector.tensor_tensor(samp[:], samp[:], tmp[:], add)
                    nc.vector.tensor_tensor(tmp[:], ga[:, :, 3], bcw(3), mult)
                    nc.vector.tensor_tensor(samp[:], samp[:], tmp2[:], add)
                    nc.vector.tensor_tensor(samp[:], samp[:], tmp[:], add)

                # transpose samp[q, (k c)] -> rhs[(k c), b, cs]
                sf = samp[:].rearrange("p k c -> p (k c)")
                for b in range(NB):
                    mlo, mhi = 128 * b, min(128 * (b + 1), CIN * K2)
                    pst = tpool.tile([128, 128], BF16, tag="tps")
                    nc.tensor.transpose(
                        pst[:mhi - mlo, :], sf[:, mlo:mhi], identb[:])
                    nc.any.tensor_copy(rhs[:mhi - mlo, b, cs],
                                       pst[:mhi - mlo, :PCH])

                # column-slice matmuls: psWB[:, cs] = [W~ ; B~] @ s_ch
                for b in range(NB):
                    nc.tensor.matmul(
                        out=psWB[:, cs], lhsT=wwb_sb[:, b, :],
                        rhs=rhs[:, b, cs],
                        start=(b == 0), stop=(b == NB - 1))
                nc.vector.tensor_tensor(out_sb[:, cs], psWB[:COUT, cs],
                                        xc_sb[:COUT, cs], mult)
                nc.vector.tensor_tensor(out_sb[:, cs], out_sb[:, cs],
                                        psWB[COUT:, cs], add)

            nc.sync.dma_start(out_d.ap(), out_sb[:])

    nc.compile()
    return nc


def _host_inputs(x, w_off, b_off, w_wgt, b_wgt):
    """Build the 8 per-core input dicts (layout/shard prep only)."""
    x = np.asarray(x, dtype=np.float32)
    w_off = np.asarray(w_off, dtype=np.float32)
    b_off = np.asarray(b_off, dtype=np.float32)
    w_wgt = np.asarray(w_wgt, dtype=np.float32)
    b_wgt = np.asarray(b_wgt, dtype=np.float32)

    # wwb [128, 5, 128]: lhsT chunk b = [W~.T | B~.T] on the output axis,
    # k-major contraction order m = k*64 + c.
    perm = np.arange(CIN * K2).reshape(CIN, K2).T.reshape(-1)
    wtp = np.zeros((NB * 128, COUT), dtype=np.float32)
    wtp[:576] = w_wgt.T[perm]
    btp = np.zeros((NB * 128, COUT), dtype=np.float32)
    btp[:576] = b_wgt.reshape(CIN, K2 * COUT).T[perm]
    wwb = np.concatenate([wtp.reshape(NB, 128, COUT),
                          btp.reshape(NB, 128, COUT)], axis=2)
    wwb_b = _to_bf16(np.ascontiguousarray(
        wwb.transpose(1, 0, 2).reshape(128, NB * 128)))

    mg = np.zeros((128, 8, 128), dtype=np.float32)
    q = np.arange(128)
    for gsel in range(8):
        mg[gsel * 16 + (q % 16), gsel, q] = 1.0
    mg = mg.reshape(128, 8 * 128)

    wofft = np.zeros((128, 2 * K2), dtype=np.float32)
    wofft[:CIN, :K2] = w_off[0::2].T
    wofft[:CIN, K2:] = w_off[1::2].T

    xs = np.linspace(-1.0, 1.0, W).astype(np.float32)
    ys = np.linspace(-1.0, 1.0, H).astype(np.float32)
    kx = np.linspace(-(K - 1) / (W - 1), (K - 1) / (W - 1), K).astype(np.float32)
    ky = np.linspace(-(K - 1) / (H - 1), (K - 1) / (H - 1), K).astype(np.float32)

    in_maps = []
    for c in range(NCORES):
        n, half = divmod(c, 2)
        r0 = HHALF * half
        xn = x[n]

        # 4-tap table on the clipped 31x31 grid; OOB taps are zero.
        pad = np.zeros((CIN, H + 5, W + 5), dtype=np.float32)
        pad[:, 2:2 + H, 2:2 + W] = xn
        t00 = pad[:, 0:31, 0:31]
        t01 = pad[:, 0:31, 1:32]
        t10 = pad[:, 1:32, 0:31]
        t11 = pad[:, 1:32, 1:32]
        tbl = np.stack([t00, t01, t10, t11], axis=0)  # [4, 64, 31, 31]
        tbl = tbl.transpose(2, 3, 0, 1).reshape(TBL_ROWS, 4 * CIN)
        tbl_b = _to_bf16(np.ascontiguousarray(tbl))

        xcpad = np.zeros((128, NPT), dtype=np.float32)
        xcpad[:CIN] = xn.reshape(CIN, H * W)[:, r0 * W:r0 * W + NPT]

        b2 = np.zeros((128, NCH, 2 * K2), dtype=np.float32)
        p_idx = np.arange(PCH)
        for ch in range(NCH):
            gpix = r0 * W + ch * PCH + p_idx
            row, col = gpix // W, gpix % W
            for kk in range(K2):
                kyi, kxi = divmod(kk, K)
                b2[:PCH, ch, kk] = ((xs[col] + kx[kxi] + b_off[2 * kk] + 1.0)
                                    * SC + 2.0)
                b2[:PCH, ch, K2 + kk] = ((ys[row] + ky[kyi] + b_off[2 * kk + 1]
                                          + 1.0) * SC + 2.0)
        b2[PCH:] = SC + 2.0

        in_maps.append({
            "tbl4": tbl_b,
            "xcpad": xcpad,
            "wofft": wofft,
            "base2": b2.reshape(128, NCH * 2 * K2),
            "wwb": wwb_b,
            "mg": mg,
        })
    return in_maps


def _to_bf16(a):
    try:
        import ml_dtypes
        return a.astype(ml_dtypes.bfloat16)
    except ImportError:
        b = a.view(np.uint32)
        rounded = ((b + 0x7FFF + ((b >> 16) & 1)) >> 16).astype(np.uint16)
        return rounded.view(np.uint16)


def get_program():
    if "nc" not in _CACHE:
        _CACHE["nc"] = _build_program()
    return _CACHE["nc"]


def run_cores(in_maps, **kw):
    nc = get_program()
    return run_bass_kernel_spmd(nc, in_maps, core_ids=list(range(NCORES)), **kw)


def assemble(results):
    out = np.zeros((N, COUT, H, W), dtype=np.float32)
    for c in range(NCORES):
        n, half = divmod(c, 2)
        out[n, :, HHALF * half:HHALF * (half + 1), :] = \
            results[c]["out"].reshape(COUT, HHALF, W)
    return out


def kernel(x, w_off, b_off, w_wgt, b_wgt):
    in_maps = _host_inputs(x, w_off, b_off, w_wgt, b_wgt)
    res = run_cores(in_maps)
    return assemble(res.results)


# revision 11
# speedup vs baseline: 2.3772x; 1.0208x over previous
"""Trainium2 Bass kernel for nn_DeformRouting (deformable routing conv), v3.

Strategy (8 cores, data-parallel over N x H-halves):
  core c handles image n = c//2, row-half = c%2 (14 rows x 28 cols = 392 pixels).

v3 structure (v2 was 88us, baseline 181us):
  - 4-tap bf16 table rows (zero-padded 31x31 grid) -> ONE gather index per
    sample, validity masking folded into table zeros.
  - Per-chunk pipeline (4 chunks of 98 points): gather -> combine ->
    transpose -> column-slice matmuls, so only the LAST chunk's tail
    (drain+combine+transpose+matmul) is exposed behind the serial
    descriptor-generation (the dominant cost, ~8ns/idx of gpsimd ucode).
  - f32r (tf32-mode) offset-conv + index-permutation matmuls (values are
    small integers -> exact; avoids fp32's 2-instruction split).
  - Input DMAs spread across engine queues.
  - Last chunk's combine splits muls DVE/gpsimd (gpsimd is idle once the
    last gather's descriptors are generated).
  - k-major contraction (m = k*64+c, host-permuted weights): contiguous
    bf16 combine writes; W~ and B~ stacked in one [128,128] lhsT chunk so 5
    matmuls per column-slice produce [W~@s ; B~@s] together.
"""

import numpy as np

import concourse.bass as bass
import concourse.tile as tile
from concourse import bacc, mybir
from concourse.bass_utils import run_bass_kernel_spmd
from concourse.masks import make_identity

N, CIN, COUT, H, W, K = 4, 64, 64, 28, 28, 3
K2 = K * K
NCORES = 8
HHALF = H // 2          # 14 rows per core
NPT = HHALF * W         # 392 points per core
PCH = 98                # points per partition-chunk
NCH = 4                 # chunks
TBL_ROWS = 31 * 31      # 961 4-tap table rows
SC = (W - 1) / 2.0      # 13.5
NI_CH = 128 * K2        # 1152 gathered rows per chunk
NB = 5                  # m-chunks of 128 (576 -> 640)

F32 = mybir.dt.float32
F32R = mybir.dt.float32r
I32 = mybir.dt.int32
BF16 = mybir.dt.bfloat16

_CACHE = {}


def _alu(name):
    return getattr(mybir.AluOpType, name)


def _build_program():
    nc = bacc.Bacc("TRN2", target_bir_lowering=False, debug=False,
                   num_devices=NCORES)

    tbl4 = nc.dram_tensor("tbl4", [TBL_ROWS, 4 * CIN], BF16, kind="ExternalInput")
    xcpad = nc.dram_tensor("xcpad", [128, NPT], F32, kind="ExternalInput")
    wofft = nc.dram_tensor("wofft", [128, 2 * K2], F32, kind="ExternalInput")
    base2 = nc.dram_tensor("base2", [128, NCH * 2 * K2], F32, kind="ExternalInput")
    wwb = nc.dram_tensor("wwb", [128, NB * 128], BF16, kind="ExternalInput")
    mg = nc.dram_tensor("mg", [128, 8 * 128], F32, kind="ExternalInput")
    out_d = nc.dram_tensor("out", [COUT, NPT], F32, kind="ExternalOutput")

    mult, add, sub = _alu("mult"), _alu("add"), _alu("subtract")
    is_gt = _alu("is_gt")
    amin, amax = _alu("min"), _alu("max")

    with tile.TileContext(nc) as tc:
        with (
            tc.tile_pool(name="const", bufs=1) as cpool,
            tc.tile_pool(name="work", bufs=1) as wpool,
            tc.tile_pool(name="psoff", bufs=1, space="PSUM") as opool,
            tc.tile_pool(name="pst", bufs=2, space="PSUM") as tpool,
            tc.tile_pool(name="pso", bufs=1, space="PSUM") as popool,
        ):
            # ---- inputs, spread across engine DMA queues ----
            xc_sb = cpool.tile([128, NPT], F32)
            nc.sync.dma_start(xc_sb[:], xcpad.ap())
            wofft_sb = cpool.tile([128, 2 * K2], F32)
            nc.scalar.dma_start(wofft_sb[:], wofft.ap())
            base2_sb = cpool.tile([128, NCH, 2 * K2], F32)
            nc.sync.dma_start(base2_sb[:], base2.ap().rearrange(
                "p (a b) -> p a b", a=NCH))
            wwb_sb = cpool.tile([128, NB, 128], BF16)
            nc.scalar.dma_start(wwb_sb[:], wwb.ap().rearrange(
                "p (a b) -> p a b", a=NB))
            mg_sb = cpool.tile([128, 8, 128], F32)
            nc.sync.dma_start(mg_sb[:], mg.ap().rearrange(
                "p (a b) -> p a b", a=8))
            identb = cpool.tile([128, 128], BF16)
            make_identity(nc, identb[:])

            # ---- 1. offset conv (f32r; pad partitions zeroed: garbage
            # would flow into gather indices past the clip) ----
            ps_off = opool.tile([128, NCH, 2 * K2], F32)
            nc.vector.memset(ps_off[:], 0.0)
            for ch in range(NCH):
                nc.tensor.matmul(
                    out=ps_off[:PCH, ch, :],
                    lhsT=xc_sb[:, ch * PCH:(ch + 1) * PCH],
                    rhs=wofft_sb[:],
                    start=True, stop=True,
                )

            # ---- 2. coordinate math on fused x|y tiles [128, NCH, 18] ----
            shp2 = [128, NCH, 2 * K2]
            _cnt = [0]

            def t(shape=shp2, dt=F32):
                _cnt[0] += 1
                return wpool.tile(shape, dt, name=f"ct{_cnt[0]}")

            ic = t()
            nc.vector.scalar_tensor_tensor(ic[:], ps_off[:], SC, base2_sb[:],
                                           mult, add)
            ti = t(dt=I32)
            nc.any.tensor_copy(ti[:], ic[:])
            tf = t()
            nc.any.tensor_copy(tf[:], ti[:])
            g = t()
            nc.vector.tensor_tensor(g[:], tf[:], ic[:], is_gt)
            f0 = t()
            nc.vector.tensor_tensor(f0[:], tf[:], g[:], sub)
            w1 = t()
            nc.vector.tensor_tensor(w1[:], ic[:], f0[:], sub)
            w0 = t()
            nc.vector.tensor_scalar(w0[:], w1[:], -1.0, 1.0, mult, add)
            cc = t()
            nc.vector.tensor_scalar(cc[:], f0[:], 30.0, 0.0, amin, amax)
            idxf = t([128, NCH, K2])
            nc.vector.scalar_tensor_tensor(idxf[:], cc[:, :, K2:], 31.0,
                                           cc[:, :, :K2], mult, add)
            w4 = t([128, 4, NCH, K2], BF16)
            nc.vector.tensor_tensor(w4[:, 0], w0[:, :, K2:], w0[:, :, :K2], mult)
            nc.vector.tensor_tensor(w4[:, 1], w0[:, :, K2:], w1[:, :, :K2], mult)
            nc.vector.tensor_tensor(w4[:, 2], w1[:, :, K2:], w0[:, :, :K2], mult)
            nc.vector.tensor_tensor(w4[:, 3], w1[:, :, K2:], w1[:, :, :K2], mult)

            # ---- 3. idx wrap: 8 f32r permutation matmuls + int16 copy ----
            psw = opool.tile([128, 8, NCH * K2], F32, name="psw")
            for gsel in range(8):
                nc.tensor.matmul(
                    out=psw[:, gsel, :], lhsT=mg_sb[:, gsel, :],
                    rhs=idxf[:].rearrange("p a b -> p (a b)"),
                    start=True, stop=True)
            wrap = wpool.tile([128, NCH, K2, 8], mybir.dt.int16, name="wrap")
            nc.vector.tensor_copy(
                wrap[:].rearrange("q a m g -> q g (a m)"), psw[:])

            # ---- 4..7 per-chunk pipeline ----
            psWB = popool.tile([128, NPT], F32, name="psWB")
            rhs = wpool.tile([128, NB, NPT], BF16)
            nc.vector.memset(rhs[64:, NB - 1, :], 0.0)
            out_sb = wpool.tile([COUT, NPT], F32)

            for ch in range(NCH):
                cs = slice(ch * PCH, (ch + 1) * PCH)
                tc.tile_set_cur_wait(0.010 * ch)
                ga = wpool.tile([128, K2, 4, CIN], BF16, name=f"ga{ch}")
                nc.gpsimd.dma_gather(
                    out_ap=ga[:].rearrange("p k t c -> p k (t c)"),
                    in_ap=tbl4.ap(),
                    idxs_ap=wrap[:, ch].rearrange("q m g -> q (m g)"),
                    num_idxs=NI_CH, num_idxs_reg=NI_CH,
                    elem_size=4 * CIN, single_packet=False)

                def bcw(tap):
                    return w4[:, tap, ch][:, :, None].to_broadcast(
                        [128, K2, CIN])

                samp = wpool.tile([128, K2, CIN], BF16, name=f"samp{ch}")
                tmp = wpool.tile([128, K2, CIN], BF16, name=f"tmp{ch}")
                if ch < NCH - 1:
                    nc.vector.tensor_tensor(samp[:], ga[:, :, 0], bcw(0), mult)
                    nc.vector.tensor_tensor(tmp[:], ga[:, :, 1], bcw(1), mult)
                    nc.vector.tensor_tensor(samp[:], samp[:], tmp[:], add)
                    nc.vector.tensor_tensor(tmp[:], ga[:, :, 2], bcw(2), mult)
                    nc.vector.tensor_tensor(samp[:], samp[:], tmp[:], add)
                    nc.vector.tensor_tensor(tmp[:], ga[:, :, 3], bcw(3), mult)
                    nc.vector.tensor_tensor(samp[:], samp[:], tmp[:], add)
                else:
                    # last chunk: gpsimd is idle after its descriptor gen --
                    # split the muls so the exposed tail shrinks
                    tmp2 = wpool.tile([128, K2, CIN], BF16, name="tmpg")
                    nc.vector.tensor_tensor(samp[:], ga[:, :, 0], bcw(0), mult)
                    nc.vector.tensor_tensor(tmp[:], ga[:, :, 1], bcw(1), mult)
                    nc.gpsimd.tensor_tensor(tmp2[:], ga[:, :, 2], bcw(2), mult)
                    nc.vector.tensor_tensor(samp[:], samp[:], tmp[:], add)
                    nc.vector.tensor_tensor(tmp[:], ga[:, :, 3], bcw(3), mult)
                    nc.vector.tensor_tensor(samp[:], samp[:], tmp2[:], add)
                    nc.vector.tensor_tensor(samp[:], samp[:], tmp[:], add)

                # transpose samp[q, (k c)] -> rhs[(k c), b, cs]
                sf = samp[:].rearrange("p k c -> p (k c)")
                for b in range(NB):
                    mlo, mhi = 128 * b, min(128 * (b + 1), CIN * K2)
                    pst = tpool.tile([128, 128], BF16, tag="tps")
                    nc.tensor.transpose(
                        pst[:mhi - mlo, :], sf[:, mlo:mhi], identb[:])
                    nc.any.tensor_copy(rhs[:mhi - mlo, b, cs],
                                       pst[:mhi - mlo, :PCH])

                # column-slice matmuls: psWB[:, cs] = [W~ ; B~] @ s_ch
                for b in range(NB):
                    nc.tensor.matmul(
                        out=psWB[:, cs], lhsT=wwb_sb[:, b, :],
                        rhs=rhs[:, b, cs],
                        start=(b == 0), stop=(b == NB - 1))
                nc.vector.tensor_tensor(out_sb[:, cs], psWB[:COUT, cs],
                                        xc_sb[:COUT, cs], mult)
                nc.vector.tensor_tensor(out_sb[:, cs], out_sb[:, cs],
                                        psWB[COUT:, cs], add)
                eng = nc.sync if ch % 2 == 0 else nc.scalar
                eng.dma_start(out_d.ap()[:, cs], out_sb[:, cs])

    nc.compile()
    return nc


def _host_inputs(x, w_off, b_off, w_wgt, b_wgt):
    """Build the 8 per-core input dicts (layout/shard prep only)."""
    x = np.asarray(x, dtype=np.float32)
    w_off = np.asarray(w_off, dtype=np.float32)
    b_off = np.asarray(b_off, dtype=np.float32)
    w_wgt = np.asarray(w_wgt, dtype=np.float32)
    b_wgt = np.asarray(b_wgt, dtype=np.float32)

    # wwb [128, 5, 128]: lhsT chunk b = [W~.T | B~.T] on the output axis,
    # k-major contraction order m = k*64 + c.
    perm = np.arange(CIN * K2).reshape(CIN, K2).T.reshape(-1)
    wtp = np.zeros((NB * 128, COUT), dtype=np.float32)
    wtp[:576] = w_wgt.T[perm]
    btp = np.zeros((NB * 128, COUT), dtype=np.float32)
    btp[:576] = b_wgt.reshape(CIN, K2 * COUT).T[perm]
    wwb = np.concatenate([wtp.reshape(NB, 128, COUT),
                          btp.reshape(NB, 128, COUT)], axis=2)
    wwb_b = _to_bf16(np.ascontiguousarray(
        wwb.transpose(1, 0, 2).reshape(128, NB * 128)))

    mg = np.zeros((128, 8, 128), dtype=np.float32)
    q = np.arange(128)
    for gsel in range(8):
        mg[gsel * 16 + (q % 16), gsel, q] = 1.0
    mg = mg.reshape(128, 8 * 128)

    wofft = np.zeros((128, 2 * K2), dtype=np.float32)
    wofft[:CIN, :K2] = w_off[0::2].T
    wofft[:CIN, K2:] = w_off[1::2].T

    xs = np.linspace(-1.0, 1.0, W).astype(np.float32)
    ys = np.linspace(-1.0, 1.0, H).astype(np.float32)
    kx = np.linspace(-(K - 1) / (W - 1), (K - 1) / (W - 1), K).astype(np.float32)
    ky = np.linspace(-(K - 1) / (H - 1), (K - 1) / (H - 1), K).astype(np.float32)

    in_maps = []
    for c in range(NCORES):
        n, half = divmod(c, 2)
        r0 = HHALF * half
        xn = x[n]

        # 4-tap table on the clipped 31x31 grid; OOB taps are zero.
        pad = np.zeros((CIN, H + 5, W + 5), dtype=np.float32)
        pad[:, 2:2 + H, 2:2 + W] = xn
        t00 = pad[:, 0:31, 0:31]
        t01 = pad[:, 0:31, 1:32]
        t10 = pad[:, 1:32, 0:31]
        t11 = pad[:, 1:32, 1:32]
        tbl = np.stack([t00, t01, t10, t11], axis=0)  # [4, 64, 31, 31]
        tbl = tbl.transpose(2, 3, 0, 1).reshape(TBL_ROWS, 4 * CIN)
        tbl_b = _to_bf16(np.ascontiguousarray(tbl))

        xcpad = np.zeros((128, NPT), dtype=np.float32)
        xcpad[:CIN] = xn.reshape(CIN, H * W)[:, r0 * W:r0 * W + NPT]

        b2 = np.zeros((128, NCH, 2 * K2), dtype=np.float32)
        p_idx = np.arange(PCH)
        for ch in range(NCH):
            gpix = r0 * W + ch * PCH + p_idx
            row, col = gpix // W, gpix % W
            for kk in range(K2):
                kyi, kxi = divmod(kk, K)
                b2[:PCH, ch, kk] = ((xs[col] + kx[kxi] + b_off[2 * kk] + 1.0)
                                    * SC + 2.0)
                b2[:PCH, ch, K2 + kk] = ((ys[row] + ky[kyi] + b_off[2 * kk + 1]
                                          + 1.0) * SC + 2.0)
        b2[PCH:] = SC + 2.0

        in_maps.append({
            "tbl4": tbl_b,
            "xcpad": xcpad,
            "wofft": wofft,
            "base2": b2.reshape(128, NCH * 2 * K2),
            "wwb": wwb_b,
            "mg": mg,
        })
    return in_maps


def _to_bf16(a):
    try:
        import ml_dtypes
        return a.astype(ml_dtypes.bfloat16)
    except ImportError:
        b = a.view(np.uint32)
        rounded = ((b + 0x7FFF + ((b >> 16) & 1)) >> 16).astype(np.uint16)
        return rounded.view(np.uint16)


def get_program():
    if "nc" not in _CACHE:
        _CACHE["nc"] = _build_program()
    return _CACHE["nc"]


def run_cores(in_maps, **kw):
    nc = get_program()
    return run_bass_kernel_spmd(nc, in_maps, core_ids=list(range(NCORES)), **kw)


def assemble(results):
    out = np.zeros((N, COUT, H, W), dtype=np.float32)
    for c in range(NCORES):
        n, half = divmod(c, 2)
        out[n, :, HHALF * half:HHALF * (half + 1), :] = \
            results[c]["out"].reshape(COUT, HHALF, W)
    return out


def kernel(x, w_off, b_off, w_wgt, b_wgt):
    in_maps = _host_inputs(x, w_off, b_off, w_wgt, b_wgt)
    res = run_cores(in_maps)
    return assemble(res.results)


# revision 12
# speedup vs baseline: 2.4837x; 1.0448x over previous
"""Trainium2 Bass kernel for nn_DeformRouting (deformable routing conv), v3.

Strategy (8 cores, data-parallel over N x H-halves):
  core c handles image n = c//2, row-half = c%2 (14 rows x 28 cols = 392 pixels).

v3 structure (v2 was 88us, baseline 181us):
  - 4-tap bf16 table rows (zero-padded 31x31 grid) -> ONE gather index per
    sample, validity masking folded into table zeros.
  - Per-chunk pipeline (4 chunks of 98 points): gather -> combine ->
    transpose -> column-slice matmuls, so only the LAST chunk's tail
    (drain+combine+transpose+matmul) is exposed behind the serial
    descriptor-generation (the dominant cost, ~8ns/idx of gpsimd ucode).
  - f32r (tf32-mode) offset-conv + index-permutation matmuls (values are
    small integers -> exact; avoids fp32's 2-instruction split).
  - Input DMAs spread across engine queues.
  - Last chunk's combine splits muls DVE/gpsimd (gpsimd is idle once the
    last gather's descriptors are generated).
  - k-major contraction (m = k*64+c, host-permuted weights): contiguous
    bf16 combine writes; W~ and B~ stacked in one [128,128] lhsT chunk so 5
    matmuls per column-slice produce [W~@s ; B~@s] together.
"""

import numpy as np

import concourse.bass as bass
import concourse.tile as tile
from concourse import bacc, mybir
from concourse.bass_utils import run_bass_kernel_spmd
from concourse.masks import make_identity

N, CIN, COUT, H, W, K = 4, 64, 64, 28, 28, 3
K2 = K * K
NCORES = 8
HHALF = H // 2          # 14 rows per core
NPT = HHALF * W         # 392 points per core
PCH = 98                # points per partition-chunk
NCH = 4                 # chunks
TBL_ROWS = 31 * 31      # 961 4-tap table rows
SC = (W - 1) / 2.0      # 13.5
NI_CH = 128 * K2        # 1152 gathered rows per chunk
NB = 5                  # m-chunks of 128 (576 -> 640)

F32 = mybir.dt.float32
F32R = mybir.dt.float32r
I32 = mybir.dt.int32
BF16 = mybir.dt.bfloat16

_CACHE = {}


def _alu(name):
    return getattr(mybir.AluOpType, name)


def _build_program():
    nc = bacc.Bacc("TRN2", target_bir_lowering=False, debug=False,
                   num_devices=NCORES)

    tbl4 = nc.dram_tensor("tbl4", [TBL_ROWS, 4 * CIN], BF16, kind="ExternalInput")
    xcpad = nc.dram_tensor("xcpad", [128, NPT], F32, kind="ExternalInput")
    wofft = nc.dram_tensor("wofft", [128, 2 * K2], F32, kind="ExternalInput")
    base2 = nc.dram_tensor("base2", [128, NCH * 2 * K2], F32, kind="ExternalInput")
    wwb = nc.dram_tensor("wwb", [128, NB * 128], BF16, kind="ExternalInput")
    mg = nc.dram_tensor("mg", [128, 8 * 128], BF16, kind="ExternalInput")
    out_d = nc.dram_tensor("out", [COUT, NPT], F32, kind="ExternalOutput")

    mult, add, sub = _alu("mult"), _alu("add"), _alu("subtract")
    is_gt = _alu("is_gt")
    amin, amax = _alu("min"), _alu("max")

    with tile.TileContext(nc) as tc:
        with (
            tc.tile_pool(name="const", bufs=1) as cpool,
            tc.tile_pool(name="work", bufs=1) as wpool,
            tc.tile_pool(name="psoff", bufs=1, space="PSUM") as opool,
            tc.tile_pool(name="pst", bufs=2, space="PSUM") as tpool,
            tc.tile_pool(name="pso", bufs=1, space="PSUM") as popool,
        ):
            # ---- inputs, spread across engine DMA queues ----
            xc_sb = cpool.tile([128, NPT], F32)
            nc.sync.dma_start(xc_sb[:], xcpad.ap())
            wofft_sb = cpool.tile([128, 2 * K2], F32)
            nc.scalar.dma_start(wofft_sb[:], wofft.ap())
            base2_sb = cpool.tile([128, NCH, 2 * K2], F32)
            nc.sync.dma_start(base2_sb[:], base2.ap().rearrange(
                "p (a b) -> p a b", a=NCH))
            wwb_sb = cpool.tile([128, NB, 128], BF16)
            nc.scalar.dma_start(wwb_sb[:], wwb.ap().rearrange(
                "p (a b) -> p a b", a=NB))
            mg_sb = cpool.tile([128, 8, 128], BF16)
            nc.sync.dma_start(mg_sb[:], mg.ap().rearrange(
                "p (a b) -> p a b", a=8))
            identb = cpool.tile([128, 128], BF16)
            make_identity(nc, identb[:])

            # ---- 1. offset conv (f32r; pad partitions zeroed: garbage
            # would flow into gather indices past the clip) ----
            ps_off = opool.tile([128, NCH, 2 * K2], F32)
            nc.vector.memset(ps_off[:], 0.0)
            for ch in range(NCH):
                nc.tensor.matmul(
                    out=ps_off[:PCH, ch, :],
                    lhsT=xc_sb[:, ch * PCH:(ch + 1) * PCH],
                    rhs=wofft_sb[:],
                    start=True, stop=True,
                )

            # ---- 2. coordinate math on fused x|y tiles [128, NCH, 18] ----
            shp2 = [128, NCH, 2 * K2]
            _cnt = [0]

            def t(shape=shp2, dt=F32):
                _cnt[0] += 1
                return wpool.tile(shape, dt, name=f"ct{_cnt[0]}")

            ic = t()
            nc.vector.scalar_tensor_tensor(ic[:], ps_off[:], SC, base2_sb[:],
                                           mult, add)
            ti = t(dt=I32)
            nc.any.tensor_copy(ti[:], ic[:])
            tf = t()
            nc.any.tensor_copy(tf[:], ti[:])
            g = t()
            nc.vector.tensor_tensor(g[:], tf[:], ic[:], is_gt)
            f0 = t()
            nc.vector.tensor_tensor(f0[:], tf[:], g[:], sub)
            w1 = t()
            nc.vector.tensor_tensor(w1[:], ic[:], f0[:], sub)
            w0 = t()
            nc.vector.tensor_scalar(w0[:], w1[:], -1.0, 1.0, mult, add)
            cc = t(dt=BF16)
            nc.vector.tensor_scalar(cc[:], f0[:], 30.0, 0.0, amin, amax)
            w4 = t([128, 4, NCH, K2], BF16)
            nc.vector.tensor_tensor(w4[:, 0], w0[:, :, K2:], w0[:, :, :K2], mult)
            nc.vector.tensor_tensor(w4[:, 1], w0[:, :, K2:], w1[:, :, :K2], mult)
            nc.vector.tensor_tensor(w4[:, 2], w1[:, :, K2:], w0[:, :, :K2], mult)
            nc.vector.tensor_tensor(w4[:, 3], w1[:, :, K2:], w1[:, :, :K2], mult)

            # ---- 3. idx wrap: permute clipped coords (bf16-exact ints),
            # then fuse row = 31*cy + cx into the int16 wrap build ----
            psw = opool.tile([128, 8, NCH, 2, K2], F32, name="psw")
            for gsel in range(8):
                nc.tensor.matmul(
                    out=psw[:, gsel].rearrange("p a x b -> p (a x b)"),
                    lhsT=mg_sb[:, gsel, :],
                    rhs=cc[:].rearrange("p a b -> p (a b)"),
                    start=True, stop=True)
            wrapf = wpool.tile([128, 8, NCH, K2], F32, name="wrapf")
            nc.vector.tensor_scalar(wrapf[:], psw[:, :, :, 1, :], 31.0, 0.0,
                                    mult, add)
            wrap = wpool.tile([128, NCH, K2, 8], mybir.dt.int16, name="wrap")
            nc.vector.tensor_tensor(
                wrap[:].rearrange("q a m g -> q g a m"), wrapf[:],
                psw[:, :, :, 0, :], add)

            # ---- 4..7 per-chunk pipeline ----
            psWB = popool.tile([128, NPT], F32, name="psWB")
            rhs = wpool.tile([128, NB, NPT], BF16)
            nc.vector.memset(rhs[64:, NB - 1, :], 0.0)
            out_sb = wpool.tile([COUT, NPT], F32)

            for ch in range(NCH):
                cs = slice(ch * PCH, (ch + 1) * PCH)
                tc.tile_set_cur_wait(0.010 * ch)
                ga = wpool.tile([128, K2, 4, CIN], BF16, name=f"ga{ch}")
                nc.gpsimd.dma_gather(
                    out_ap=ga[:].rearrange("p k t c -> p k (t c)"),
                    in_ap=tbl4.ap(),
                    idxs_ap=wrap[:, ch].rearrange("q m g -> q (m g)"),
                    num_idxs=NI_CH, num_idxs_reg=NI_CH,
                    elem_size=4 * CIN, single_packet=False)

                def bcw(tap):
                    return w4[:, tap, ch][:, :, None].to_broadcast(
                        [128, K2, CIN])

                samp = wpool.tile([128, K2, CIN], BF16, name=f"samp{ch}")
                tmp = wpool.tile([128, K2, CIN], BF16, name=f"tmp{ch}")
                nc.vector.tensor_tensor(samp[:], ga[:, :, 0], bcw(0), mult)
                nc.vector.tensor_tensor(tmp[:], ga[:, :, 1], bcw(1), mult)
                nc.vector.tensor_tensor(samp[:], samp[:], tmp[:], add)
                nc.vector.tensor_tensor(tmp[:], ga[:, :, 2], bcw(2), mult)
                nc.vector.tensor_tensor(samp[:], samp[:], tmp[:], add)
                nc.vector.tensor_tensor(tmp[:], ga[:, :, 3], bcw(3), mult)
                nc.vector.tensor_tensor(samp[:], samp[:], tmp[:], add)

                # transpose samp[q, (k c)] -> rhs[(k c), b, cs]
                sf = samp[:].rearrange("p k c -> p (k c)")
                for b in range(NB):
                    mlo, mhi = 128 * b, min(128 * (b + 1), CIN * K2)
                    pst = tpool.tile([128, 128], BF16, tag="tps")
                    nc.tensor.transpose(
                        pst[:mhi - mlo, :], sf[:, mlo:mhi], identb[:])
                    nc.any.tensor_copy(rhs[:mhi - mlo, b, cs],
                                       pst[:mhi - mlo, :PCH])

                # column-slice matmuls: psWB[:, cs] = [W~ ; B~] @ s_ch
                for b in range(NB):
                    nc.tensor.matmul(
                        out=psWB[:, cs], lhsT=wwb_sb[:, b, :],
                        rhs=rhs[:, b, cs],
                        start=(b == 0), stop=(b == NB - 1))
                nc.vector.tensor_tensor(out_sb[:, cs], psWB[:COUT, cs],
                                        xc_sb[:COUT, cs], mult)
                nc.vector.tensor_tensor(out_sb[:, cs], out_sb[:, cs],
                                        psWB[COUT:, cs], add)
                eng = nc.sync if ch % 2 == 0 else nc.scalar
                eng.dma_start(out_d.ap()[:, cs], out_sb[:, cs])

    nc.compile()
    return nc


def _host_inputs(x, w_off, b_off, w_wgt, b_wgt):
    """Build the 8 per-core input dicts (layout/shard prep only)."""
    x = np.asarray(x, dtype=np.float32)
    w_off = np.asarray(w_off, dtype=np.float32)
    b_off = np.asarray(b_off, dtype=np.float32)
    w_wgt = np.asarray(w_wgt, dtype=np.float32)
    b_wgt = np.asarray(b_wgt, dtype=np.float32)

    # wwb [128, 5, 128]: lhsT chunk b = [W~.T | B~.T] on the output axis,
    # k-major contraction order m = k*64 + c.
    perm = np.arange(CIN * K2).reshape(CIN, K2).T.reshape(-1)
    wtp = np.zeros((NB * 128, COUT), dtype=np.float32)
    wtp[:576] = w_wgt.T[perm]
    btp = np.zeros((NB * 128, COUT), dtype=np.float32)
    btp[:576] = b_wgt.reshape(CIN, K2 * COUT).T[perm]
    wwb = np.concatenate([wtp.reshape(NB, 128, COUT),
                          btp.reshape(NB, 128, COUT)], axis=2)
    wwb_b = _to_bf16(np.ascontiguousarray(
        wwb.transpose(1, 0, 2).reshape(128, NB * 128)))

    mg = np.zeros((128, 8, 128), dtype=np.float32)
    q = np.arange(128)
    for gsel in range(8):
        mg[gsel * 16 + (q % 16), gsel, q] = 1.0
    mg = _to_bf16(mg.reshape(128, 8 * 128))

    wofft = np.zeros((128, 2 * K2), dtype=np.float32)
    wofft[:CIN, :K2] = w_off[0::2].T
    wofft[:CIN, K2:] = w_off[1::2].T

    xs = np.linspace(-1.0, 1.0, W).astype(np.float32)
    ys = np.linspace(-1.0, 1.0, H).astype(np.float32)
    kx = np.linspace(-(K - 1) / (W - 1), (K - 1) / (W - 1), K).astype(np.float32)
    ky = np.linspace(-(K - 1) / (H - 1), (K - 1) / (H - 1), K).astype(np.float32)

    in_maps = []
    for c in range(NCORES):
        n, half = divmod(c, 2)
        r0 = HHALF * half
        xn = x[n]

        # 4-tap table on the clipped 31x31 grid; OOB taps are zero.
        pad = np.zeros((CIN, H + 5, W + 5), dtype=np.float32)
        pad[:, 2:2 + H, 2:2 + W] = xn
        t00 = pad[:, 0:31, 0:31]
        t01 = pad[:, 0:31, 1:32]
        t10 = pad[:, 1:32, 0:31]
        t11 = pad[:, 1:32, 1:32]
        tbl = np.stack([t00, t01, t10, t11], axis=0)  # [4, 64, 31, 31]
        tbl = tbl.transpose(2, 3, 0, 1).reshape(TBL_ROWS, 4 * CIN)
        tbl_b = _to_bf16(np.ascontiguousarray(tbl))

        xcpad = np.zeros((128, NPT), dtype=np.float32)
        xcpad[:CIN] = xn.reshape(CIN, H * W)[:, r0 * W:r0 * W + NPT]

        b2 = np.zeros((128, NCH, 2 * K2), dtype=np.float32)
        p_idx = np.arange(PCH)
        for ch in range(NCH):
            gpix = r0 * W + ch * PCH + p_idx
            row, col = gpix // W, gpix % W
            for kk in range(K2):
                kyi, kxi = divmod(kk, K)
                b2[:PCH, ch, kk] = ((xs[col] + kx[kxi] + b_off[2 * kk] + 1.0)
                                    * SC + 2.0)
                b2[:PCH, ch, K2 + kk] = ((ys[row] + ky[kyi] + b_off[2 * kk + 1]
                                          + 1.0) * SC + 2.0)
        b2[PCH:] = SC + 2.0

        in_maps.append({
            "tbl4": tbl_b,
            "xcpad": xcpad,
            "wofft": wofft,
            "base2": b2.reshape(128, NCH * 2 * K2),
            "wwb": wwb_b,
            "mg": mg,
        })
    return in_maps


def _to_bf16(a):
    try:
        import ml_dtypes
        return a.astype(ml_dtypes.bfloat16)
    except ImportError:
        b = a.view(np.uint32)
        rounded = ((b + 0x7FFF + ((b >> 16) & 1)) >> 16).astype(np.uint16)
        return rounded.view(np.uint16)


def get_program():
    if "nc" not in _CACHE:
        _CACHE["nc"] = _build_program()
    return _CACHE["nc"]


def run_cores(in_maps, **kw):
    nc = get_program()
    return run_bass_kernel_spmd(nc, in_maps, core_ids=list(range(NCORES)), **kw)


def assemble(results):
    out = np.zeros((N, COUT, H, W), dtype=np.float32)
    for c in range(NCORES):
        n, half = divmod(c, 2)
        out[n, :, HHALF * half:HHALF * (half + 1), :] = \
            results[c]["out"].reshape(COUT, HHALF, W)
    return out


def kernel(x, w_off, b_off, w_wgt, b_wgt):
    in_maps = _host_inputs(x, w_off, b_off, w_wgt, b_wgt)
    res = run_cores(in_maps)
    return assemble(res.results)


# revision 13
# speedup vs baseline: 2.5249x; 1.0166x over previous
"""Trainium2 Bass kernel for nn_DeformRouting (deformable routing conv), v3.

Strategy (8 cores, data-parallel over N x H-halves):
  core c handles image n = c//2, row-half = c%2 (14 rows x 28 cols = 392 pixels).

v3 structure (v2 was 88us, baseline 181us):
  - 4-tap bf16 table rows (zero-padded 31x31 grid) -> ONE gather index per
    sample, validity masking folded into table zeros.
  - Per-chunk pipeline (4 chunks of 98 points): gather -> combine ->
    transpose -> column-slice matmuls, so only the LAST chunk's tail
    (drain+combine+transpose+matmul) is exposed behind the serial
    descriptor-generation (the dominant cost, ~8ns/idx of gpsimd ucode).
  - f32r (tf32-mode) offset-conv + index-permutation matmuls (values are
    small integers -> exact; avoids fp32's 2-instruction split).
  - Input DMAs spread across engine queues.
  - Last chunk's combine splits muls DVE/gpsimd (gpsimd is idle once the
    last gather's descriptors are generated).
  - k-major contraction (m = k*64+c, host-permuted weights): contiguous
    bf16 combine writes; W~ and B~ stacked in one [128,128] lhsT chunk so 5
    matmuls per column-slice produce [W~@s ; B~@s] together.
"""

import numpy as np

import concourse.bass as bass
import concourse.tile as tile
from concourse import bacc, mybir
from concourse.bass_utils import run_bass_kernel_spmd
from concourse.masks import make_identity

N, CIN, COUT, H, W, K = 4, 64, 64, 28, 28, 3
K2 = K * K
NCORES = 8
HHALF = H // 2          # 14 rows per core
NPT = HHALF * W         # 392 points per core
PCH = 98                # points per partition-chunk
NCH = 4                 # chunks
TBL_ROWS = 31 * 31      # 961 4-tap table rows
SC = (W - 1) / 2.0      # 13.5
NI_CH = 128 * K2        # 1152 gathered rows per chunk
NB = 5                  # m-chunks of 128 (576 -> 640)

F32 = mybir.dt.float32
F32R = mybir.dt.float32r
I32 = mybir.dt.int32
BF16 = mybir.dt.bfloat16

_CACHE = {}


def _alu(name):
    return getattr(mybir.AluOpType, name)


def _build_program():
    nc = bacc.Bacc("TRN2", target_bir_lowering=False, debug=False,
                   num_devices=NCORES)

    tbl4 = nc.dram_tensor("tbl4", [TBL_ROWS, 4 * CIN], BF16, kind="ExternalInput")
    xcpad = nc.dram_tensor("xcpad", [128, NPT], F32, kind="ExternalInput")
    wofft = nc.dram_tensor("wofft", [128, 2 * K2], F32, kind="ExternalInput")
    base2 = nc.dram_tensor("base2", [128, NCH * 2 * K2], F32, kind="ExternalInput")
    wwb = nc.dram_tensor("wwb", [128, NB * 128], BF16, kind="ExternalInput")
    mg = nc.dram_tensor("mg", [128, 8 * 128], BF16, kind="ExternalInput")
    out_d = nc.dram_tensor("out", [COUT, NPT], F32, kind="ExternalOutput")

    mult, add, sub = _alu("mult"), _alu("add"), _alu("subtract")
    is_gt = _alu("is_gt")
    amin, amax = _alu("min"), _alu("max")

    with tile.TileContext(nc) as tc:
        with (
            tc.tile_pool(name="const", bufs=1) as cpool,
            tc.tile_pool(name="work", bufs=1) as wpool,
            tc.tile_pool(name="psoff", bufs=1, space="PSUM") as opool,
            tc.tile_pool(name="pst", bufs=2, space="PSUM") as tpool,
            tc.tile_pool(name="pso", bufs=1, space="PSUM") as popool,
        ):
            # ---- inputs, spread across engine DMA queues ----
            xc_sb = cpool.tile([128, NPT], F32)
            nc.sync.dma_start(xc_sb[:], xcpad.ap())
            wofft_sb = cpool.tile([128, 2 * K2], F32)
            nc.scalar.dma_start(wofft_sb[:], wofft.ap())
            base2_sb = cpool.tile([128, NCH, 2 * K2], F32)
            nc.sync.dma_start(base2_sb[:], base2.ap().rearrange(
                "p (a b) -> p a b", a=NCH))
            wwb_sb = cpool.tile([128, NB, 128], BF16)
            nc.scalar.dma_start(wwb_sb[:], wwb.ap().rearrange(
                "p (a b) -> p a b", a=NB))
            mg_sb = cpool.tile([128, 8, 128], BF16)
            nc.sync.dma_start(mg_sb[:], mg.ap().rearrange(
                "p (a b) -> p a b", a=8))
            identb = cpool.tile([128, 128], BF16)
            make_identity(nc, identb[:])

            # ---- 1. offset conv (f32r; pad partitions zeroed: garbage
            # would flow into gather indices past the clip) ----
            ps_off = opool.tile([128, NCH, 2 * K2], F32)
            nc.vector.memset(ps_off[:], 0.0)
            for ch in range(NCH):
                nc.tensor.matmul(
                    out=ps_off[:PCH, ch, :],
                    lhsT=xc_sb[:, ch * PCH:(ch + 1) * PCH],
                    rhs=wofft_sb[:],
                    start=True, stop=True,
                )

            # ---- 2. coordinate math on fused x|y tiles [128, NCH, 18] ----
            shp2 = [128, NCH, 2 * K2]
            _cnt = [0]

            def t(shape=shp2, dt=F32):
                _cnt[0] += 1
                return wpool.tile(shape, dt, name=f"ct{_cnt[0]}")

            ic = t()
            nc.vector.scalar_tensor_tensor(ic[:], ps_off[:], SC, base2_sb[:],
                                           mult, add)
            ti = t(dt=I32)
            nc.any.tensor_copy(ti[:], ic[:])
            tf = t()
            nc.any.tensor_copy(tf[:], ti[:])
            g = t()
            nc.vector.tensor_tensor(g[:], tf[:], ic[:], is_gt)
            f0 = t()
            nc.vector.tensor_tensor(f0[:], tf[:], g[:], sub)
            w1 = t()
            nc.vector.tensor_tensor(w1[:], ic[:], f0[:], sub)
            w0 = t()
            nc.vector.tensor_scalar(w0[:], w1[:], -1.0, 1.0, mult, add)
            cc = t(dt=BF16)
            nc.vector.tensor_scalar(cc[:], f0[:], 30.0, 0.0, amin, amax)
            w4 = t([128, 4, NCH, K2], BF16)
            nc.vector.tensor_tensor(w4[:, 0], w0[:, :, K2:], w0[:, :, :K2], mult)
            nc.vector.tensor_tensor(w4[:, 1], w0[:, :, K2:], w1[:, :, :K2], mult)
            nc.vector.tensor_tensor(w4[:, 2], w1[:, :, K2:], w0[:, :, :K2], mult)
            nc.vector.tensor_tensor(w4[:, 3], w1[:, :, K2:], w1[:, :, :K2], mult)

            # ---- 3. idx wrap: permute clipped coords (bf16-exact ints),
            # then fuse row = 31*cy + cx into the int16 wrap build ----
            psw = opool.tile([128, 8, NCH, 2, K2], F32, name="psw")
            for gsel in range(8):
                nc.tensor.matmul(
                    out=psw[:, gsel].rearrange("p a x b -> p (a x b)"),
                    lhsT=mg_sb[:, gsel, :],
                    rhs=cc[:].rearrange("p a b -> p (a b)"),
                    start=True, stop=True)
            wrapf = wpool.tile([128, 8, NCH, K2], F32, name="wrapf")
            nc.vector.tensor_scalar(wrapf[:], psw[:, :, :, 1, :], 31.0, 0.0,
                                    mult, add)
            wrap = wpool.tile([128, NCH, K2, 8], mybir.dt.int16, name="wrap")
            nc.vector.tensor_tensor(
                wrap[:].rearrange("q a m g -> q g a m"), wrapf[:],
                psw[:, :, :, 0, :], add)

            # ---- 4..7 per-chunk pipeline ----
            psWB = popool.tile([128, NPT], F32, name="psWB")
            rhs = wpool.tile([128, NB, NPT], BF16)
            nc.vector.memset(rhs[64:, NB - 1, :], 0.0)
            out_sb = wpool.tile([COUT, NPT], F32)

            # pieces (ch, klo, khi): the last chunk is split so most of
            # its drain+combine hides under the second piece's desc-gen
            pieces = [(0, 0, 9), (1, 0, 9), (2, 0, 9), (3, 0, 5), (3, 5, 9)]
            samps = [wpool.tile([128, K2, CIN], BF16, name=f"samp{c}")
                     for c in range(NCH)]
            tmps = [wpool.tile([128, K2, CIN], BF16, name=f"tmp{c}")
                    for c in range(NCH)]
            for pidx, (ch, klo, khi) in enumerate(pieces):
                cs = slice(ch * PCH, (ch + 1) * PCH)
                nk = khi - klo
                tc.tile_set_cur_wait(0.009 * pidx)
                ga = wpool.tile([128, nk, 4, CIN], BF16, name=f"ga{pidx}")
                nc.gpsimd.dma_gather(
                    out_ap=ga[:].rearrange("p k t c -> p k (t c)"),
                    in_ap=tbl4.ap(),
                    idxs_ap=wrap[:, ch, klo:khi].rearrange("q m g -> q (m g)"),
                    num_idxs=128 * nk, num_idxs_reg=128 * nk,
                    elem_size=4 * CIN, single_packet=False)

                def bcw(tap):
                    return w4[:, tap, ch, klo:khi][:, :, None].to_broadcast(
                        [128, nk, CIN])

                samp = samps[ch][:, klo:khi]
                tmp = tmps[ch][:, klo:khi]
                nc.vector.tensor_tensor(samp, ga[:, :, 0], bcw(0), mult)
                nc.vector.tensor_tensor(tmp, ga[:, :, 1], bcw(1), mult)
                nc.vector.tensor_tensor(samp, samp, tmp, add)
                nc.vector.tensor_tensor(tmp, ga[:, :, 2], bcw(2), mult)
                nc.vector.tensor_tensor(samp, samp, tmp, add)
                nc.vector.tensor_tensor(tmp, ga[:, :, 3], bcw(3), mult)
                nc.vector.tensor_tensor(samp, samp, tmp, add)

                if khi < K2:
                    continue
                # transpose samp[q, (k c)] -> rhs[(k c), b, cs]
                sf = samps[ch][:].rearrange("p k c -> p (k c)")
                for b in range(NB):
                    mlo, mhi = 128 * b, min(128 * (b + 1), CIN * K2)
                    pst = tpool.tile([128, 128], BF16, tag="tps")
                    nc.tensor.transpose(
                        pst[:mhi - mlo, :], sf[:, mlo:mhi], identb[:])
                    nc.any.tensor_copy(rhs[:mhi - mlo, b, cs],
                                       pst[:mhi - mlo, :PCH])

                # column-slice matmuls: psWB[:, cs] = [W~ ; B~] @ s_ch
                for b in range(NB):
                    nc.tensor.matmul(
                        out=psWB[:, cs], lhsT=wwb_sb[:, b, :],
                        rhs=rhs[:, b, cs],
                        start=(b == 0), stop=(b == NB - 1))
                nc.vector.tensor_tensor(out_sb[:, cs], psWB[:COUT, cs],
                                        xc_sb[:COUT, cs], mult)
                nc.vector.tensor_tensor(out_sb[:, cs], out_sb[:, cs],
                                        psWB[COUT:, cs], add)
                eng = nc.sync if ch % 2 == 0 else nc.scalar
                eng.dma_start(out_d.ap()[:, cs], out_sb[:, cs])

    nc.compile()
    return nc


def _host_inputs(x, w_off, b_off, w_wgt, b_wgt):
    """Build the 8 per-core input dicts (layout/shard prep only)."""
    x = np.asarray(x, dtype=np.float32)
    w_off = np.asarray(w_off, dtype=np.float32)
    b_off = np.asarray(b_off, dtype=np.float32)
    w_wgt = np.asarray(w_wgt, dtype=np.float32)
    b_wgt = np.asarray(b_wgt, dtype=np.float32)

    # wwb [128, 5, 128]: lhsT chunk b = [W~.T | B~.T] on the output axis,
    # k-major contraction order m = k*64 + c.
    perm = np.arange(CIN * K2).reshape(CIN, K2).T.reshape(-1)
    wtp = np.zeros((NB * 128, COUT), dtype=np.float32)
    wtp[:576] = w_wgt.T[perm]
    btp = np.zeros((NB * 128, COUT), dtype=np.float32)
    btp[:576] = b_wgt.reshape(CIN, K2 * COUT).T[perm]
    wwb = np.concatenate([wtp.reshape(NB, 128, COUT),
                          btp.reshape(NB, 128, COUT)], axis=2)
    wwb_b = _to_bf16(np.ascontiguousarray(
        wwb.transpose(1, 0, 2).reshape(128, NB * 128)))

    mg = np.zeros((128, 8, 128), dtype=np.float32)
    q = np.arange(128)
    for gsel in range(8):
        mg[gsel * 16 + (q % 16), gsel, q] = 1.0
    mg = _to_bf16(mg.reshape(128, 8 * 128))

    wofft = np.zeros((128, 2 * K2), dtype=np.float32)
    wofft[:CIN, :K2] = w_off[0::2].T
    wofft[:CIN, K2:] = w_off[1::2].T

    xs = np.linspace(-1.0, 1.0, W).astype(np.float32)
    ys = np.linspace(-1.0, 1.0, H).astype(np.float32)
    kx = np.linspace(-(K - 1) / (W - 1), (K - 1) / (W - 1), K).astype(np.float32)
    ky = np.linspace(-(K - 1) / (H - 1), (K - 1) / (H - 1), K).astype(np.float32)

    in_maps = []
    for c in range(NCORES):
        n, half = divmod(c, 2)
        r0 = HHALF * half
        xn = x[n]

        # 4-tap table on the clipped 31x31 grid; OOB taps are zero.
        pad = np.zeros((CIN, H + 5, W + 5), dtype=np.float32)
        pad[:, 2:2 + H, 2:2 + W] = xn
        t00 = pad[:, 0:31, 0:31]
        t01 = pad[:, 0:31, 1:32]
        t10 = pad[:, 1:32, 0:31]
        t11 = pad[:, 1:32, 1:32]
        tbl = np.stack([t00, t01, t10, t11], axis=0)  # [4, 64, 31, 31]
        tbl = tbl.transpose(2, 3, 0, 1).reshape(TBL_ROWS, 4 * CIN)
        tbl_b = _to_bf16(np.ascontiguousarray(tbl))

        xcpad = np.zeros((128, NPT), dtype=np.float32)
        xcpad[:CIN] = xn.reshape(CIN, H * W)[:, r0 * W:r0 * W + NPT]

        b2 = np.zeros((128, NCH, 2 * K2), dtype=np.float32)
        p_idx = np.arange(PCH)
        for ch in range(NCH):
            gpix = r0 * W + ch * PCH + p_idx
            row, col = gpix // W, gpix % W
            for kk in range(K2):
                kyi, kxi = divmod(kk, K)
                b2[:PCH, ch, kk] = ((xs[col] + kx[kxi] + b_off[2 * kk] + 1.0)
                                    * SC + 2.0)
                b2[:PCH, ch, K2 + kk] = ((ys[row] + ky[kyi] + b_off[2 * kk + 1]
                                          + 1.0) * SC + 2.0)
        b2[PCH:] = SC + 2.0

        in_maps.append({
            "tbl4": tbl_b,
            "xcpad": xcpad,
            "wofft": wofft,
            "base2": b2.reshape(128, NCH * 2 * K2),
            "wwb": wwb_b,
            "mg": mg,
        })
    return in_maps


def _to_bf16(a):
    try:
        import ml_dtypes
        return a.astype(ml_dtypes.bfloat16)
    except ImportError:
        b = a.view(np.uint32)
        rounded = ((b + 0x7FFF + ((b >> 16) & 1)) >> 16).astype(np.uint16)
        return rounded.view(np.uint16)


def get_program():
    if "nc" not in _CACHE:
        _CACHE["nc"] = _build_program()
    return _CACHE["nc"]


def run_cores(in_maps, **kw):
    nc = get_program()
    return run_bass_kernel_spmd(nc, in_maps, core_ids=list(range(NCORES)), **kw)


def assemble(results):
    out = np.zeros((N, COUT, H, W), dtype=np.float32)
    for c in range(NCORES):
        n, half = divmod(c, 2)
        out[n, :, HHALF * half:HHALF * (half + 1), :] = \
            results[c]["out"].reshape(COUT, HHALF, W)
    return out


def kernel(x, w_off, b_off, w_wgt, b_wgt):
    in_maps = _host_inputs(x, w_off, b_off, w_wgt, b_wgt)
    res = run_cores(in_maps)
    return assemble(res.results)


# revision 14
# speedup vs baseline: 2.5496x; 1.0098x over previous
"""Trainium2 Bass kernel for nn_DeformRouting (deformable routing conv), v3.

Strategy (8 cores, data-parallel over N x H-halves):
  core c handles image n = c//2, row-half = c%2 (14 rows x 28 cols = 392 pixels).

Pipeline (baseline 181us -> 72us):
  - 4-tap bf16 table rows (zero-padded 31x31 grid) -> ONE gather index per
    sample; validity masking folded into table zeros.
  - Per-piece pipeline over 5 gather pieces (9/9/9/5/4 k-slots): gather ->
    combine -> transpose -> column-slice matmuls, so only the small last
    piece's tail is exposed behind the serial SWDGE descriptor generation
    (the dominant cost, ~8ns/idx of gpsimd ucode; ~39us total).
  - Index-permutation matmuls run in bf16 on the SPLIT clipped coords
    (cy, cx <= 30 are bf16-exact); row = 31*cy+cx is fused into the int16
    wrap build.  tile_set_cur_wait paces the scheduler so later pieces'
    combines are not hoisted ahead on the in-order DVE queue.
  - k-major contraction (m = k*64+c, host-permuted weights): contiguous
    bf16 combine writes; W~ and B~ stacked in one [128,128] lhsT chunk so 5
    matmuls per column-slice produce [W~@s ; B~@s] together; out =
    psWB[:64]*x + psWB[64:] per the grouped weight-gen algebra.
"""

import numpy as np

import concourse.bass as bass
import concourse.tile as tile
from concourse import bacc, mybir
from concourse.bass_utils import run_bass_kernel_spmd
from concourse.masks import make_identity

N, CIN, COUT, H, W, K = 4, 64, 64, 28, 28, 3
K2 = K * K
NCORES = 8
HHALF = H // 2          # 14 rows per core
NPT = HHALF * W         # 392 points per core
PCH = 98                # points per partition-chunk
NCH = 4                 # chunks
TBL_ROWS = 31 * 31      # 961 4-tap table rows
SC = (W - 1) / 2.0      # 13.5
NI_CH = 128 * K2        # 1152 gathered rows per chunk
NB = 5                  # m-chunks of 128 (576 -> 640)

F32 = mybir.dt.float32
F32R = mybir.dt.float32r
I32 = mybir.dt.int32
BF16 = mybir.dt.bfloat16

_CACHE = {}


def _alu(name):
    return getattr(mybir.AluOpType, name)


def _build_program():
    nc = bacc.Bacc("TRN2", target_bir_lowering=False, debug=False,
                   num_devices=NCORES)

    tbl4 = nc.dram_tensor("tbl4", [TBL_ROWS, 4 * CIN], BF16, kind="ExternalInput")
    xcpad = nc.dram_tensor("xcpad", [128, NPT], F32, kind="ExternalInput")
    wofft = nc.dram_tensor("wofft", [128, 2 * K2], F32, kind="ExternalInput")
    base2 = nc.dram_tensor("base2", [128, NCH * 2 * K2], F32, kind="ExternalInput")
    wwb = nc.dram_tensor("wwb", [128, NB * 128], BF16, kind="ExternalInput")
    mg = nc.dram_tensor("mg", [128, 8 * 128], BF16, kind="ExternalInput")
    out_d = nc.dram_tensor("out", [COUT, NPT], F32, kind="ExternalOutput")

    mult, add, sub = _alu("mult"), _alu("add"), _alu("subtract")
    is_gt = _alu("is_gt")
    amin, amax = _alu("min"), _alu("max")

    with tile.TileContext(nc) as tc:
        with (
            tc.tile_pool(name="const", bufs=1) as cpool,
            tc.tile_pool(name="work", bufs=1) as wpool,
            tc.tile_pool(name="psoff", bufs=1, space="PSUM") as opool,
            tc.tile_pool(name="pst", bufs=2, space="PSUM") as tpool,
            tc.tile_pool(name="pso", bufs=1, space="PSUM") as popool,
        ):
            # ---- inputs, spread across engine DMA queues ----
            xc_sb = cpool.tile([128, NPT], F32)
            nc.sync.dma_start(xc_sb[:], xcpad.ap())
            wofft_sb = cpool.tile([128, 2 * K2], F32)
            nc.scalar.dma_start(wofft_sb[:], wofft.ap())
            base2_sb = cpool.tile([128, NCH, 2 * K2], F32)
            nc.sync.dma_start(base2_sb[:], base2.ap().rearrange(
                "p (a b) -> p a b", a=NCH))
            wwb_sb = cpool.tile([128, NB, 128], BF16)
            nc.scalar.dma_start(wwb_sb[:], wwb.ap().rearrange(
                "p (a b) -> p a b", a=NB))
            mg_sb = cpool.tile([128, 8, 128], BF16)
            nc.sync.dma_start(mg_sb[:], mg.ap().rearrange(
                "p (a b) -> p a b", a=8))
            identb = cpool.tile([128, 128], BF16)
            make_identity(nc, identb[:])

            # ---- 1. offset conv (pad partitions zeroed: garbage would
            # flow into gather indices past the clip) ----
            ps_off = opool.tile([128, NCH, 2 * K2], F32)
            nc.vector.memset(ps_off[:], 0.0)
            for ch in range(NCH):
                nc.tensor.matmul(
                    out=ps_off[:PCH, ch, :],
                    lhsT=xc_sb[:, ch * PCH:(ch + 1) * PCH],
                    rhs=wofft_sb[:],
                    start=True, stop=True,
                )

            # ---- 2. coordinate math on fused x|y tiles [128, NCH, 18] ----
            shp2 = [128, NCH, 2 * K2]
            _cnt = [0]

            def t(shape=shp2, dt=F32):
                _cnt[0] += 1
                return wpool.tile(shape, dt, name=f"ct{_cnt[0]}")

            ic = t()
            nc.vector.scalar_tensor_tensor(ic[:], ps_off[:], SC, base2_sb[:],
                                           mult, add)
            ti = t(dt=I32)
            nc.any.tensor_copy(ti[:], ic[:])
            tf = t()
            nc.any.tensor_copy(tf[:], ti[:])
            g = t()
            nc.vector.tensor_tensor(g[:], tf[:], ic[:], is_gt)
            f0 = t()
            nc.vector.tensor_tensor(f0[:], tf[:], g[:], sub)
            w1 = t()
            nc.vector.tensor_tensor(w1[:], ic[:], f0[:], sub)
            w0 = t()
            nc.vector.tensor_scalar(w0[:], w1[:], -1.0, 1.0, mult, add)
            cc = t(dt=BF16)
            nc.vector.tensor_scalar(cc[:], f0[:], 30.0, 0.0, amin, amax)
            w4 = t([128, 4, NCH, K2], BF16)
            nc.vector.tensor_tensor(w4[:, 0], w0[:, :, K2:], w0[:, :, :K2], mult)
            nc.vector.tensor_tensor(w4[:, 1], w0[:, :, K2:], w1[:, :, :K2], mult)
            nc.vector.tensor_tensor(w4[:, 2], w1[:, :, K2:], w0[:, :, :K2], mult)
            nc.vector.tensor_tensor(w4[:, 3], w1[:, :, K2:], w1[:, :, :K2], mult)

            # ---- 3. idx wrap: permute clipped coords (bf16-exact ints),
            # then fuse row = 31*cy + cx into the int16 wrap build ----
            psw = opool.tile([128, 8, NCH, 2, K2], F32, name="psw")
            for gsel in range(8):
                nc.tensor.matmul(
                    out=psw[:, gsel].rearrange("p a x b -> p (a x b)"),
                    lhsT=mg_sb[:, gsel, :],
                    rhs=cc[:].rearrange("p a b -> p (a b)"),
                    start=True, stop=True)
            wrapf = wpool.tile([128, 8, NCH, K2], F32, name="wrapf")
            nc.vector.tensor_scalar(wrapf[:], psw[:, :, :, 1, :], 31.0, 0.0,
                                    mult, add)
            wrap = wpool.tile([128, NCH, K2, 8], mybir.dt.int16, name="wrap")
            nc.vector.tensor_tensor(
                wrap[:].rearrange("q a m g -> q g a m"), wrapf[:],
                psw[:, :, :, 0, :], add)

            # ---- 4..7 per-chunk pipeline ----
            psWB = popool.tile([128, NPT], F32, name="psWB")
            rhs = wpool.tile([128, NB, NPT], BF16)
            nc.vector.memset(rhs[64:, NB - 1, :], 0.0)
            out_sb = wpool.tile([COUT, NPT], F32)

            # pieces (ch, klo, khi): the last chunk is split so most of
            # its drain+combine hides under the second piece's desc-gen
            pieces = [(0, 0, 9), (1, 0, 9), (2, 0, 9), (3, 0, 5), (3, 5, 9)]
            samps = [wpool.tile([128, K2, CIN], BF16, name=f"samp{c}")
                     for c in range(NCH)]
            tmps = [wpool.tile([128, K2, CIN], BF16, name=f"tmp{c}")
                    for c in range(NCH)]
            for pidx, (ch, klo, khi) in enumerate(pieces):
                cs = slice(ch * PCH, (ch + 1) * PCH)
                nk = khi - klo
                tc.tile_set_cur_wait(0.009 * pidx)
                ga = wpool.tile([128, nk, 4, CIN], BF16, name=f"ga{pidx}")
                nc.gpsimd.dma_gather(
                    out_ap=ga[:].rearrange("p k t c -> p k (t c)"),
                    in_ap=tbl4.ap(),
                    idxs_ap=wrap[:, ch, klo:khi].rearrange("q m g -> q (m g)"),
                    num_idxs=128 * nk, num_idxs_reg=128 * nk,
                    elem_size=4 * CIN, single_packet=False)

                def bcw(tap):
                    return w4[:, tap, ch, klo:khi][:, :, None].to_broadcast(
                        [128, nk, CIN])

                samp = samps[ch][:, klo:khi]
                tmp = tmps[ch][:, klo:khi]
                nc.vector.tensor_tensor(samp, ga[:, :, 0], bcw(0), mult)
                nc.vector.tensor_tensor(tmp, ga[:, :, 1], bcw(1), mult)
                nc.vector.tensor_tensor(samp, samp, tmp, add)
                nc.vector.tensor_tensor(tmp, ga[:, :, 2], bcw(2), mult)
                nc.vector.tensor_tensor(samp, samp, tmp, add)
                nc.vector.tensor_tensor(tmp, ga[:, :, 3], bcw(3), mult)
                nc.vector.tensor_tensor(samp, samp, tmp, add)

                if khi < K2:
                    continue
                # transpose samp[q, (k c)] -> rhs[(k c), b, cs]
                sf = samps[ch][:].rearrange("p k c -> p (k c)")
                for b in range(NB):
                    mlo, mhi = 128 * b, min(128 * (b + 1), CIN * K2)
                    pst = tpool.tile([128, 128], BF16, tag="tps")
                    nc.tensor.transpose(
                        pst[:mhi - mlo, :], sf[:, mlo:mhi], identb[:])
                    nc.any.tensor_copy(rhs[:mhi - mlo, b, cs],
                                       pst[:mhi - mlo, :PCH])

                # column-slice matmuls: psWB[:, cs] = [W~ ; B~] @ s_ch
                for b in range(NB):
                    nc.tensor.matmul(
                        out=psWB[:, cs], lhsT=wwb_sb[:, b, :],
                        rhs=rhs[:, b, cs],
                        start=(b == 0), stop=(b == NB - 1))
                nc.vector.tensor_tensor(out_sb[:, cs], psWB[:COUT, cs],
                                        xc_sb[:COUT, cs], mult)
                nc.vector.tensor_tensor(out_sb[:, cs], out_sb[:, cs],
                                        psWB[COUT:, cs], add)
                eng = nc.sync if ch % 2 == 0 else nc.scalar
                eng.dma_start(out_d.ap()[:, cs], out_sb[:, cs])

    nc.compile()
    return nc


def _host_inputs(x, w_off, b_off, w_wgt, b_wgt):
    """Build the 8 per-core input dicts (layout/shard prep only)."""
    x = np.asarray(x, dtype=np.float32)
    w_off = np.asarray(w_off, dtype=np.float32)
    b_off = np.asarray(b_off, dtype=np.float32)
    w_wgt = np.asarray(w_wgt, dtype=np.float32)
    b_wgt = np.asarray(b_wgt, dtype=np.float32)

    # wwb [128, 5, 128]: lhsT chunk b = [W~.T | B~.T] on the output axis,
    # k-major contraction order m = k*64 + c.
    perm = np.arange(CIN * K2).reshape(CIN, K2).T.reshape(-1)
    wtp = np.zeros((NB * 128, COUT), dtype=np.float32)
    wtp[:576] = w_wgt.T[perm]
    btp = np.zeros((NB * 128, COUT), dtype=np.float32)
    btp[:576] = b_wgt.reshape(CIN, K2 * COUT).T[perm]
    wwb = np.concatenate([wtp.reshape(NB, 128, COUT),
                          btp.reshape(NB, 128, COUT)], axis=2)
    wwb_b = _to_bf16(np.ascontiguousarray(
        wwb.transpose(1, 0, 2).reshape(128, NB * 128)))

    mg = np.zeros((128, 8, 128), dtype=np.float32)
    q = np.arange(128)
    for gsel in range(8):
        mg[gsel * 16 + (q % 16), gsel, q] = 1.0
    mg = _to_bf16(mg.reshape(128, 8 * 128))

    wofft = np.zeros((128, 2 * K2), dtype=np.float32)
    wofft[:CIN, :K2] = w_off[0::2].T
    wofft[:CIN, K2:] = w_off[1::2].T

    xs = np.linspace(-1.0, 1.0, W).astype(np.float32)
    ys = np.linspace(-1.0, 1.0, H).astype(np.float32)
    kx = np.linspace(-(K - 1) / (W - 1), (K - 1) / (W - 1), K).astype(np.float32)
    ky = np.linspace(-(K - 1) / (H - 1), (K - 1) / (H - 1), K).astype(np.float32)

    in_maps = []
    for c in range(NCORES):
        n, half = divmod(c, 2)
        r0 = HHALF * half
        xn = x[n]

        # 4-tap table on the clipped 31x31 grid; OOB taps are zero.
        pad = np.zeros((CIN, H + 5, W + 5), dtype=np.float32)
        pad[:, 2:2 + H, 2:2 + W] = xn
        t00 = pad[:, 0:31, 0:31]
        t01 = pad[:, 0:31, 1:32]
        t10 = pad[:, 1:32, 0:31]
        t11 = pad[:, 1:32, 1:32]
        tbl = np.stack([t00, t01, t10, t11], axis=0)  # [4, 64, 31, 31]
        tbl = tbl.transpose(2, 3, 0, 1).reshape(TBL_ROWS, 4 * CIN)
        tbl_b = _to_bf16(np.ascontiguousarray(tbl))

        xcpad = np.zeros((128, NPT), dtype=np.float32)
        xcpad[:CIN] = xn.reshape(CIN, H * W)[:, r0 * W:r0 * W + NPT]

        b2 = np.zeros((128, NCH, 2 * K2), dtype=np.float32)
        p_idx = np.arange(PCH)
        for ch in range(NCH):
            gpix = r0 * W + ch * PCH + p_idx
            row, col = gpix // W, gpix % W
            for kk in range(K2):
                kyi, kxi = divmod(kk, K)
                b2[:PCH, ch, kk] = ((xs[col] + kx[kxi] + b_off[2 * kk] + 1.0)
                                    * SC + 2.0)
                b2[:PCH, ch, K2 + kk] = ((ys[row] + ky[kyi] + b_off[2 * kk + 1]
                                          + 1.0) * SC + 2.0)
        b2[PCH:] = SC + 2.0

        in_maps.append({
            "tbl4": tbl_b,
            "xcpad": xcpad,
            "wofft": wofft,
            "base2": b2.reshape(128, NCH * 2 * K2),
            "wwb": wwb_b,
            "mg": mg,
        })
    return in_maps


def _to_bf16(a):
    try:
        import ml_dtypes
        return a.astype(ml_dtypes.bfloat16)
    except ImportError:
        b = a.view(np.uint32)
        rounded = ((b + 0x7FFF + ((b >> 16) & 1)) >> 16).astype(np.uint16)
        return rounded.view(np.uint16)


def get_program():
    if "nc" not in _CACHE:
        _CACHE["nc"] = _build_program()
    return _CACHE["nc"]


def run_cores(in_maps, **kw):
    nc = get_program()
    return run_bass_kernel_spmd(nc, in_maps, core_ids=list(range(NCORES)), **kw)


def assemble(results):
    out = np.zeros((N, COUT, H, W), dtype=np.float32)
    for c in range(NCORES):
        n, half = divmod(c, 2)
        out[n, :, HHALF * half:HHALF * (half + 1), :] = \
            results[c]["out"].reshape(COUT, HHALF, W)
    return out


def kernel(x, w_off, b_off, w_wgt, b_wgt):
    in_maps = _host_inputs(x, w_off, b_off, w_wgt, b_wgt)
    res = run_cores(in_maps)
    return assemble(res.results)
